# revision 21
# baseline (speedup 1.0000x reference)
"""Trainium2 Bass kernel for nn_AttentionFusion (dense transformer block).

Data-parallel over batch: B=8192 rows sharded as 1024 rows per NeuronCore
across 8 cores; weights replicated. On-chip layout is feature-major:
activations are stored as [128 partitions(features), k_tiles, 1024 rows],
so every matmul is out.T[m,n] = sum_k W.T[k,m] * act.T[k,n] with natural
(host-pre-transposed) weight loads and the contraction on the partition dim.

Algebraic simplifications (validated against the reference to 2e-6):
  - Cross-attention has seq len 1 -> softmax == 1 -> out = v @ wo.T + bo;
    additionally (v @ wv.T) @ wo.T = v @ (wo@wv).T is merged on the host.
  - Self-attention has seq len 2 -> softmax([a,b]) = [sig(a-b), 1-sig(a-b)].
  - LayerNorm / attention-score reductions over features (= partitions) are
    done with small matmuls against ones/head-mask matrices.

fp8 (e4m3) DoubleRow matmuls (2x PE throughput, validated vs numpy model):
  - SA q/k/v, SA out-proj, FFN w1/w2 run as fp8 DoubleRow (contract 256/instr).
  - Weights are host-quantized at 8x scale (12x for the second FFN position to
    decorrelate quantization noise between the two pooled positions);
    activations are stored as value/8 in fp8, so PSUM results come out at true
    scale and evictions keep their 1-op baseline form.
  - Cross-attention / gate / input projections stay bf16 (noise budget).
"""

import numpy as np
import ml_dtypes

import concourse.bacc as bacc
import concourse.mybir as mybir
import concourse.tile as tile
from concourse.bass_utils import run_bass_kernel_spmd

AF = mybir.ActivationFunctionType
ALU = mybir.AluOpType
BF16 = mybir.dt.bfloat16
F32 = mybir.dt.float32
FP8 = mybir.dt.float8e4
DR = mybir.MatmulPerfMode.DoubleRow

N_CORES = 8
B, IMG_D, TXT_D, H, NH = 8192, 1280, 2048, 1024, 16
HD = H // NH  # 64 head dim
R = B // N_CORES  # 1024 rows per core
P = 128
CH = 2  # row chunks per core
CHS = R // CH  # 512 rows per chunk
KT_I, KT_T, KT_H, KT_F = IMG_D // P, TXT_D // P, H // P, 4 * H // P
EPS = 1e-5
SA = 8.0    # fp8 weight scale, position 0 / shared
SB = 12.0   # fp8 weight scale, position 1 (FFN dual-quantization)

np_bf16 = ml_dtypes.bfloat16
np_fp8 = ml_dtypes.float8_e4m3


def _chsl(ch):
    return slice(ch * CHS, (ch + 1) * CHS)


def build():
    nc = bacc.Bacc(None, target_bir_lowering=False)

    def din(name, shape, dt=BF16):
        return nc.dram_tensor(name, shape, dt, kind="ExternalInput")

    xiT = din("xiT", [IMG_D, R])
    xtT = din("xtT", [TXT_D, R])
    wiT = din("wiT", [IMG_D, H])
    wtT = din("wtT", [TXT_D, H])
    wvoT = din("wvoT", [H, H])
    wqT = din("wqT", [H, H], FP8)
    wqbT = din("wqbT", [H, H], FP8)
    wkT = din("wkT", [H, H], FP8)
    wvT = din("wvT", [H, H], FP8)
    wvbT = din("wvbT", [H, H], FP8)
    woT = din("woT", [H, H], FP8)
    w1aT = din("w1aT", [H, 4 * H], FP8)
    w1bT = din("w1bT", [H, 4 * H], FP8)
    w2aT = din("w2aT", [4 * H, H], FP8)
    w2bT = din("w2bT", [4 * H, H], FP8)
    gwiT = din("gwiT", [H, H])
    gwtT = din("gwtT", [H, H])

    bias_names = ["bi", "bt", "bvo", "sbq", "sbv8", "sbo", "fb2", "gb",
                  "n1g", "n1b", "n2g", "n2b", "n3gf", "n3bf",
                  "n1g8", "n1b8", "n2g8", "n2b8", "n3g8", "n3b8"]
    NB = len(bias_names) * KT_H + KT_F
    bias_all_d = din("bias_all", [P, NB], F32)
    hmask_d = din("hmask", [P, 2])
    hmaskT_d = din("hmaskT", [2, P])

    # DRAM spill for imgp/txtp between P3 and P8 (frees SBUF during SA/FFN)
    imgp_d = nc.dram_tensor("imgp_spill", [P, KT_H, R], BF16)
    txtp_d = nc.dram_tensor("txtp_spill", [P, KT_H, R], BF16)

    outT = nc.dram_tensor("outT", [H, R], F32, kind="ExternalOutput")

    with tile.TileContext(nc) as tc:
        def open_pool(**kw):
            cm = tc.tile_pool(**kw)
            return cm, cm.__enter__()

        def scope(name):
            import contextlib

            @contextlib.contextmanager
            def _s():
                sid, _ = nc.enter_named_scope(name, False)
                yield
                nc.leave_named_scope(name, sid, False)
            return _s()

        # -------- constants (whole kernel) --------
        const_cm, const = open_pool(name="const", bufs=1)
        ones128 = const.tile([P, P], BF16)
        nc.vector.memset(ones128[:], 1.0)
        eps_col = const.tile([P, 1], F32)
        nc.vector.memset(eps_col[:], EPS)
        zero_col = const.tile([P, 1], F32)
        nc.vector.memset(zero_col[:], 0.0)
        bias_all = const.tile([P, NB], F32)
        bias_sb = {n: bias_all[:, i * KT_H:(i + 1) * KT_H]
                   for i, n in enumerate(bias_names)}
        fb1_sb = bias_all[:, len(bias_names) * KT_H:]
        hmask_sb = const.tile([P, 2], BF16)
        hmaskT_sb = const.tile([2, P], BF16)

        def load_consts():
            nc.sync.dma_start(bias_all[:], bias_all_d[:, :])
            nc.sync.dma_start(hmask_sb[:], hmask_d[:, :])
            nc.sync.dma_start(hmaskT_sb[:], hmaskT_d[:, :])

        # -------- shared SBUF pools (whole kernel) --------
        wpool_cm, wpool = open_pool(name="wpool", bufs=2)   # "w" 4KB slots x2
        tpool_cm, tpool = open_pool(name="tpool", bufs=6)   # "tmp" 2KB x6
        spool_cm, spool = open_pool(name="spool", bufs=4)   # "small" 2KB x4
        acts_cm, acts = open_pool(name="acts", bufs=1)

        def act_tile(tag, name, dt=BF16, pad16=True):
            shape = [P, KT_H, R]
            pad = None
            if dt == FP8 and pad16:
                pad = [P, KT_H, 2 * R]  # keep the recycled slot at 16KB
            return acts.tile(shape, dt, tag=tag, name=name, padded_shape=pad)

        def sp_tile(name, shape=None, dt=BF16):
            """SP slot is 16KB/partition (bf16 [P, KT_H, R])."""
            shape = shape or [P, KT_H, R]
            pad = None
            if mybir.dt.size(dt) == 1:
                pad = [shape[0], shape[1], shape[2] * 2]
            return acts.tile(shape, dt, tag="SP", name=name, padded_shape=pad)

        pmain = None
        paux = None

        def load_w(wT_d, kt, mt, name):
            """bf16 weight slice wT[:, mt*128:(mt+1)*128] as [128, kt, 128]."""
            if kt > KT_H:
                t = wpool.tile([P, KT_F, P], BF16, tag="w", name=name)
            else:
                t = wpool.tile([P, KT_H, P], BF16, tag="w_h", name=name, bufs=6)
            nc.sync.dma_start(
                t[:, :kt, :],
                wT_d[:, mt * P:(mt + 1) * P].rearrange("(k p) m -> p k m", p=P),
            )
            return t

        def load_w8(wT_d, kt, mt, name):
            """fp8 weight slice as [128, kt, 128] (big 'w' slot is 4KB fp8)."""
            if kt > KT_H:
                t = wpool.tile([P, KT_F, P], FP8, tag="w", name=name)
            else:
                t = wpool.tile([P, KT_H, P], FP8, tag="w_h", name=name, bufs=6,
                               padded_shape=[P, KT_H, 2 * P])
            nc.sync.dma_start(
                t[:, :kt, :],
                wT_d[:, mt * P:(mt + 1) * P].rearrange("(k p) m -> p k m", p=P),
            )
            return t

        def mm_layer(srcs, mt, evict, wname="w", chs=range(CH)):
            wts = [load_w(w_d, kt, mt, f"{wname}{i}") for i, (w_d, _, kt) in enumerate(srcs)]
            nk_tot = sum(kt for (_, _, kt) in srcs)
            for ch in chs:
                ps = pmain.tile([P, CHS], F32, tag="mm", name="ps_mm")
                i = 0
                for (w_d, act, kt), wt in zip(srcs, wts):
                    for k in range(kt):
                        nc.tensor.matmul(
                            ps[:], lhsT=wt[:, k, :], rhs=act[:, k, _chsl(ch)],
                            start=(i == 0), stop=(i == nk_tot - 1),
                        )
                        i += 1
                evict(mt, ch, ps)

        def mm_dr(ps, wt, act8, kt, ch, start=True, stop=True):
            """fp8 DoubleRow accumulation chain: kt k-tiles as kt//2 pairs."""
            np_ = kt // 2
            for k in range(np_):
                nc.tensor.matmul(
                    ps[:], lhsT=wt[:, 2 * k:2 * k + 2, :],
                    rhs=act8[:, 2 * k:2 * k + 2, _chsl(ch)],
                    start=(start and k == 0), stop=(stop and k == np_ - 1),
                    perf_mode=DR,
                )

        def evict_bias(dst, bname):
            b = bias_sb[bname]

            def _e(mt, ch, ps):
                nc.scalar.activation(
                    dst[:, mt, _chsl(ch)], ps[:], AF.Identity,
                    bias=b[:, mt:mt + 1], scale=1.0,
                )
            return _e

        def evict_bias_res(dst, bname, res):
            b = bias_sb[bname]

            def _e(mt, ch, ps):
                nc.vector.scalar_tensor_tensor(
                    dst[:, mt, _chsl(ch)], ps[:], b[:, mt:mt + 1],
                    res[:, mt, _chsl(ch)], op0=ALU.add, op1=ALU.add,
                )
            return _e

        lnp_cm, lnp = open_pool(name="lnp", bufs=4)  # LN stats (mf bf16, ivf f32)

        def ln_stats(x_bf, ch):
            """Row mean + rsqrt(var) via ones-matmuls. Emitted for ALL chunks
            before any normalize pass so the x^2 ACT feeds are not queued
            behind the normalize evict ACTs (which stalled the Qb matmuls)."""
            sb_ = paux.tile([P, CHS], F32, tag="Sb", name="ln_Sb")
            for k in range(KT_H):
                nc.tensor.matmul(sb_[:], lhsT=ones128[:],
                                 rhs=x_bf[:, k, _chsl(ch)],
                                 start=(k == 0), stop=(k == KT_H - 1))
            qb_ = paux.tile([P, CHS], F32, tag="Qb", name="ln_Qb")
            for k in range(KT_H):
                x2 = tpool.tile([P, CHS], BF16, tag="tmp", name="ln_x2")
                nc.vector.tensor_mul(out=x2[:], in0=x_bf[:, k, _chsl(ch)],
                                     in1=x_bf[:, k, _chsl(ch)])
                nc.tensor.matmul(qb_[:], lhsT=ones128[:], rhs=x2[:],
                                 start=(k == 0), stop=(k == KT_H - 1))
            mf = lnp.tile([P, CHS], BF16, tag="lnm", name="ln_mf")
            nc.vector.tensor_scalar_mul(mf[:], sb_[:], 1.0 / H)
            msq = tpool.tile([P, CHS], F32, tag="tmp", name="ln_msq")
            nc.vector.tensor_mul(out=msq[:], in0=mf[:], in1=mf[:])
            vf = tpool.tile([P, CHS], F32, tag="tmp", name="ln_vf")
            nc.vector.scalar_tensor_tensor(vf[:], qb_[:], 1.0 / H, msq[:],
                                           op0=ALU.mult, op1=ALU.subtract)
            sd = tpool.tile([P, CHS], F32, tag="tmp", name="ln_sd")
            nc.scalar.activation(sd[:], vf[:], AF.Sqrt, bias=eps_col[:], scale=1.0)
            # ~5x faster than nc.vector.reciprocal (which stalled PE 3.4us)
            ivf_f = tpool.tile([P, CHS], F32, tag="tmp", name="ln_ivf_f")
            nc.vector.reciprocal_approx_fast(out=ivf_f[:], in_=sd[:])
            ivf = lnp.tile([P, CHS], BF16, tag="lni", name="ln_ivf")
            nc.vector.tensor_scalar_mul(ivf[:], ivf_f[:], 1.0)
            return mf, ivf

        def ln_norm(x_bf, stats, ch, g_name, b_name, out_bf,
                    out_f8=None, g8_name=None, b8_name=None):
            g = bias_sb[g_name]
            bb = bias_sb[b_name]
            mf, ivf = stats
            for k in range(KT_H):
                t1 = tpool.tile([P, CHS], BF16, tag="tmp", name="ln_t1")
                nc.vector.tensor_sub(out=t1[:], in0=x_bf[:, k, _chsl(ch)], in1=mf[:])
                t2 = tpool.tile([P, CHS], BF16, tag="tmp", name="ln_t2")
                nc.vector.tensor_mul(out=t2[:], in0=t1[:], in1=ivf[:])
                nc.vector.tensor_scalar(out_bf[:, k, _chsl(ch)], t2[:],
                                        g[:, k:k + 1], bb[:, k:k + 1],
                                        op0=ALU.mult, op1=ALU.add)
                if out_f8 is not None:
                    g8 = bias_sb[g8_name]
                    b8 = bias_sb[b8_name]
                    nc.gpsimd.tensor_scalar(out_f8[:, k, _chsl(ch)], t2[:],
                                            g8[:, k:k + 1], b8[:, k:k + 1],
                                            op0=ALU.mult, op1=ALU.add)

        def layernorm(x_bf, g_name, b_name, out_bf,
                      out_f8=None, g8_name=None, b8_name=None):
            st = [ln_stats(x_bf, ch) for ch in range(CH)]
            for ch in range(CH):
                ln_norm(x_bf, st[ch], ch, g_name, b_name, out_bf,
                        out_f8, g8_name, b8_name)

        # ================= P0/P1: input projections (streamed) =============
        imgp = act_tile("S1", "imgp")
        txtp = act_tile("S2", "txtp")

        def input_proj(xT_d, w_d, kt_in, bname, dst, post_dma=None):
            for ch in range(CH):
                pss = [pmain.tile([P, CHS], F32, tag=f"mm{mt}", name=f"ps{mt}")
                       for mt in range(KT_H)]
                for k in range(kt_in):
                    wt = wpool.tile([P, H], BF16, tag="w_h", name="wrow", bufs=6)
                    nc.sync.dma_start(wt[:], w_d[k * P:(k + 1) * P, :])
                    xs = tpool.tile([P, CHS], BF16, tag="tmp", name="xslice")
                    nc.sync.dma_start(xs[:], xT_d[k * P:(k + 1) * P, _chsl(ch)])
                    for mt in range(KT_H):
                        nc.tensor.matmul(pss[mt][:], lhsT=wt[:, mt * P:(mt + 1) * P],
                                         rhs=xs[:], start=(k == 0), stop=(k == kt_in - 1))
                if post_dma is not None:
                    post_dma()
                    post_dma = None
                for mt in range(KT_H):
                    nc.scalar.activation(dst[:, mt, _chsl(ch)], pss[mt][:], AF.Identity,
                                         bias=bias_sb[bname][:, mt:mt + 1], scale=1.0)

        with scope("P01"), tc.tile_pool(name="pmm01", bufs=1, space="PSUM") as pmain:
            input_proj(xiT, wiT, KT_I, "bi", imgp, post_dma=load_consts)
            input_proj(xtT, wtT, KT_T, "bt", txtp)
            nc.sync.dma_start(imgp_d[:, :, :], imgp[:])
            nc.sync.dma_start(txtp_d[:, :, :], txtp[:])
            # prefetch P23's first two weight tiles into the big-weight slots
            # (sized to match the 4KB fp8 FFN slices that share the tag)
            wvo_pre = []
            for mt in range(2):
                t = wpool.tile([P, KT_H, P], BF16, tag="w", name=f"wvo_pre{mt}",
                               padded_shape=[P, 2 * KT_H, P])
                nc.sync.dma_start(
                    t[:, :, :],
                    wvoT[:, mt * P:(mt + 1) * P].rearrange("(k p) m -> p k m", p=P))
                wvo_pre.append(t)

        # ============ P2/P3: merged cross-attention + LN ============
        c0 = act_tile("S3", "c0")
        c1 = act_tile("S4", "c1")
        c0_f8 = act_tile("C8a", "c0_f8", FP8, pad16=False)  # c0/8 for DR rhs
        c1_f8 = act_tile("C8b", "c1_f8", FP8, pad16=False)

        with (
            scope("P23"),
            tc.tile_pool(name="pmm23", bufs=6, space="PSUM") as pmain,
            tc.tile_pool(name="paux23", bufs=1, space="PSUM") as paux,
            tc.tile_pool(name="pca", bufs=1) as pca,
        ):
            # GEMMs first (x0 then x1, weights loaded once per mt); both LN
            # chains afterwards so their ACT/DVE tails hide under the gate
            # GEMMs of P3g (PE never waits on LN).
            x0 = sp_tile("x0")
            ev_x0 = evict_bias_res(x0, "bvo", imgp)
            for mt in range(KT_H):
                if mt < 2:
                    for ch in range(CH):
                        ps = pmain.tile([P, CHS], F32, tag="mm", name="ps_mm")
                        for k in range(KT_H):
                            nc.tensor.matmul(ps[:], lhsT=wvo_pre[mt][:, k, :],
                                             rhs=txtp[:, k, _chsl(ch)],
                                             start=(k == 0), stop=(k == KT_H - 1))
                        ev_x0(mt, ch, ps)
                else:
                    mm_layer([(wvoT, txtp, KT_H)], mt, ev_x0, wname="wvo")
            x1 = pca.tile([P, KT_H, R], BF16, tag="x1", name="x1")
            for mt in range(KT_H):
                mm_layer([(wvoT, imgp, KT_H)], mt, evict_bias_res(x1, "bvo", txtp),
                         wname="wvo")
            st0 = [ln_stats(x0, ch) for ch in range(CH)]
            for ch in range(CH):
                ln_norm(x0, st0[ch], ch, "n1g", "n1b", c0,
                        c0_f8, "n1g8", "n1b8")
            st1 = [ln_stats(x1, ch) for ch in range(CH)]
            cd_f8 = sp_tile("cd", dt=FP8)  # (c0-c1)/8, built per tile
            for ch in range(CH):
                ln_norm(x1, st1[ch], ch, "n2g", "n2b", c1,
                        c1_f8, "n2g8", "n2b8")
                for k in range(KT_H):
                    nc.vector.tensor_sub(out=cd_f8[:, k, _chsl(ch)],
                                         in0=c0_f8[:, k, _chsl(ch)],
                                         in1=c1_f8[:, k, _chsl(ch)])

        # ================ P3g: gate logits (independent filler) ==========
        g_sb = act_tile("SG", "g_sb")
        with (
            scope("P3g"),
            tc.tile_pool(name="pmm3g", bufs=8, space="PSUM") as pmain,
        ):
            for mt in range(KT_H):
                wgi = load_w(gwiT, KT_H, mt, "wgi")
                wgt = load_w(gwtT, KT_H, mt, "wgt")
                for ch in range(CH):
                    ps = pmain.tile([P, CHS], F32, tag="mm", name="ps_g")
                    for k in range(KT_H):
                        nc.tensor.matmul(ps[:], lhsT=wgi[:, k, :],
                                         rhs=imgp[:, k, _chsl(ch)],
                                         start=(k == 0), stop=False)
                    for k in range(KT_H):
                        nc.tensor.matmul(ps[:], lhsT=wgt[:, k, :],
                                         rhs=txtp[:, k, _chsl(ch)],
                                         start=False, stop=(k == KT_H - 1))
                    if (mt + ch) % 2 == 0:
                        nc.scalar.activation(g_sb[:, mt, _chsl(ch)], ps[:], AF.Identity,
                                             bias=bias_sb["gb"][:, mt:mt + 1], scale=1.0)
                    else:
                        nc.vector.tensor_scalar(g_sb[:, mt, _chsl(ch)], ps[:],
                                                bias_sb["gb"][:, mt:mt + 1], None,
                                                op0=ALU.add)

        # ============ P4: self-attention qkv + scores (fp8 DR) ============
        v0 = act_tile("S5", "v0")   # stored as v/8 (bf16)
        v1 = act_tile("S6", "v1")
        o0 = act_tile("S1", "o0", FP8)  # o/8 in fp8, after imgp's last read
        o1 = act_tile("S2", "o1", FP8)

        with (
            scope("P4"),
            tc.tile_pool(name="pmm4", bufs=3, space="PSUM") as pmain,
            tc.tile_pool(name="pd", bufs=2, space="PSUM") as pd,
            tc.tile_pool(name="pab", bufs=1, space="PSUM") as pab,
            tc.tile_pool(name="pqk", bufs=1) as pqk,
        ):
            def qkv8(wt, act8, bname, mt, dst_t, dst_mt=None, scale=1.0):
                for ch in range(CH):
                    ps = pmain.tile([P, CHS], F32, tag="mm", name="ps_qkv")
                    mm_dr(ps, wt, act8, KT_H, ch)
                    bias = bias_sb[bname][:, mt:mt + 1] if bname else zero_col[:, :]
                    if dst_mt is None:
                        nc.scalar.activation(dst_t[:, _chsl(ch)], ps[:], AF.Identity,
                                             bias=bias, scale=scale)
                    else:
                        nc.scalar.activation(dst_t[:, dst_mt, _chsl(ch)], ps[:],
                                             AF.Identity, bias=bias, scale=scale)

            hm2 = hmask_sb[:, :]    # [128, 2] local-head one-hot
            hmT2 = hmaskT_sb[:, :]  # [2, 128]
            AB = float(SA / SB)
            for mt in range(KT_H):
                wq_t = load_w8(wqT, KT_H, mt, "wq")
                wv_t = load_w8(wvT, KT_H, mt, "wv")
                wk_t = load_w8(wkT, KT_H, mt, "wk")
                wqb_t = load_w8(wqbT, KT_H, mt, "wqb")
                wvb_t = load_w8(wvbT, KT_H, mt, "wvb")
                q0t = pqk.tile([P, R], BF16, tag="q0t")
                q1t = pqk.tile([P, R], BF16, tag="q1t")
                kdt = pqk.tile([P, R], BF16, tag="kdt")
                qkv8(wq_t, c0_f8, "sbq", mt, q0t)
                qkv8(wv_t, c0_f8, "sbv8", mt, v0, dst_mt=mt, scale=1.0 / 8.0)
                qkv8(wqb_t, c1_f8, "sbq", mt, q1t, scale=AB)
                qkv8(wk_t, cd_f8, None, mt, kdt)  # k0-k1; bias cancels
                nc.vector.tensor_mul(out=q0t[:], in0=q0t[:], in1=kdt[:])
                nc.vector.tensor_mul(out=q1t[:], in0=q1t[:], in1=kdt[:])
                m0, m1 = q0t, q1t
                a_ts = {}
                for ch in range(CH):
                    for m_t, nm in ((m0, "A"), (m1, "B")):
                        dmm = pd.tile([2, CHS], F32, tag="dmm", name=f"dmm{nm}")
                        nc.tensor.matmul(dmm[:], lhsT=hm2, rhs=m_t[:, _chsl(ch)],
                                         start=True, stop=True)
                        a_t = spool.tile([2, CHS], BF16, tag="small", name=f"a{nm}")
                        nc.scalar.activation(a_t[:], dmm[:], AF.Sigmoid,
                                             bias=zero_col[0:2, :],
                                             scale=float(1.0 / np.sqrt(HD)))
                        a_ts[(ch, nm)] = a_t
                qkv8(wvb_t, c1_f8, "sbv8", mt, v1, dst_mt=mt, scale=1.0 / SB)
                for ch in range(CH):
                    diff = tpool.tile([P, CHS], BF16, tag="tmp", name="att_diff")
                    nc.vector.tensor_sub(out=diff[:], in0=v0[:, mt, _chsl(ch)],
                                         in1=v1[:, mt, _chsl(ch)])
                    for o_t, nm in ((o0, "A"), (o1, "B")):
                        ab = pab.tile([P, CHS], F32, tag=f"ab{nm}", name=f"ab{nm}")
                        nc.tensor.matmul(ab[:], lhsT=hmT2, rhs=a_ts[(ch, nm)][:],
                                         start=True, stop=True)
                        t_t = tpool.tile([P, CHS], BF16, tag="tmp", name=f"att_t{nm}")
                        nc.vector.tensor_mul(out=t_t[:], in0=diff[:], in1=ab[:])
                        nc.vector.tensor_add(out=o_t[:, mt, _chsl(ch)], in0=t_t[:],
                                             in1=v1[:, mt, _chsl(ch)])

        # prefetch the first two FFN w1 tiles into the free big-weight slots
        # so P67 starts without a DMA wait
        w1_pre = []
        for mt in range(2):
            t = wpool.tile([P, KT_F, P], FP8, tag="w", name=f"w1_pre{mt}")
            nc.sync.dma_start(
                t[:, :KT_H, :],
                w1aT[:, mt * P:(mt + 1) * P].rearrange("(k p) m -> p k m", p=P))
            w1_pre.append(t)

        # ===== P5: SA out-proj (fp8 DR) + residual + LN3 =====
        # LN3's bf16 output is stored as r + ffn_b2 (bias n3bf = n3_b + fb2) so
        # the FFN w2 evict needs no extra bias op; the fp8 copy holds r/8.
        r0 = act_tile("S1", "r0")    # r0 + fb2 (bf16); reuses o0 slot
        r1 = act_tile("S2", "r1")
        r0_f8 = act_tile("C8a", "r0_f8", FP8, pad16=False)  # reuse c_f8 slots
        r1_f8 = act_tile("C8b", "r1_f8", FP8, pad16=False)
        with (
            scope("P5"),
            tc.tile_pool(name="pmm5", bufs=6, space="PSUM") as pmain,
            tc.tile_pool(name="paux5", bufs=1, space="PSUM") as paux,
            tc.tile_pool(name="psa", bufs=1) as psa,
        ):
            y0 = sp_tile("y0")
            y1 = psa.tile([P, KT_H, R], BF16, tag="y1", name="y1")

            # wo loaded once per mt, both positions; LN3 chains emitted after
            # all GEMMs so their ACT/DVE tails hide under LN3-y1 / P67 PE work.
            for mt in range(KT_H):
                wt = load_w8(woT, KT_H, mt, "wo")
                for o_t, res, dst in ((o0, c0, y0), (o1, c1, y1)):
                    for ch in range(CH):
                        ps = pmain.tile([P, CHS], F32, tag="mm", name="ps_wo")
                        mm_dr(ps, wt, o_t, KT_H, ch)
                        nc.vector.scalar_tensor_tensor(
                            dst[:, mt, _chsl(ch)], ps[:],
                            bias_sb["sbo"][:, mt:mt + 1],
                            res[:, mt, _chsl(ch)], op0=ALU.add, op1=ALU.add)
            sty0 = [ln_stats(y0, ch) for ch in range(CH)]
            for ch in range(CH):
                ln_norm(y0, sty0[ch], ch, "n3gf", "n3bf", r0,
                        r0_f8, "n3g8", "n3b8")
            sty1 = [ln_stats(y1, ch) for ch in range(CH)]
            for ch in range(CH):
                ln_norm(y1, sty1[ch], ch, "n3gf", "n3bf", r1,
                        r1_f8, "n3g8", "n3b8")

        # gate sigmoid in place (ACT is idle here; shortens the P8 tail)
        for mt in range(KT_H):
            for ch in range(CH):
                nc.scalar.activation(g_sb[:, mt, _chsl(ch)], g_sb[:, mt, _chsl(ch)],
                                     AF.Sigmoid, bias=zero_col[:], scale=1.0)

        imgp2 = act_tile("S3", "imgp2")
        txtp2 = act_tile("S4", "txtp2")

        # ===== P6/P7: FFN both positions (fp8 DR); pooled accumulation =====
        # mt-outer / ch-inner: each weight tile is loaded once per position
        # (halves FFN weight DMA); hidden tiles hold the full row range.
        pooled = sp_tile("pooled")  # bf16; pos1 fuses the final combine
        with (
            scope("P67"),
            tc.tile_pool(name="pmm67", bufs=8, space="PSUM") as pmain,
        ):
            gate_done = False
            for pos, (r_f8, r_p, w1d, w2d, first) in enumerate([
                    (r0_f8, r0, w1aT, w2aT, True),
                    (r1_f8, r1, w1bT, w2bT, False)]):
                if not first and not gate_done:
                    # g_sb <- gate*(imgp-txtp) + txtp in place; runs on DVE
                    # during pos0's PE work so the pos1 chain is 3 ops + DMA
                    gate_done = True
                    for gmt in range(KT_H):
                        for gch in range(CH):
                            gsl = _chsl(gch)
                            gd = tpool.tile([P, CHS], BF16, tag="tmp", name="gd")
                            nc.gpsimd.tensor_sub(out=gd[:],
                                                 in0=imgp2[:, gmt, gsl],
                                                 in1=txtp2[:, gmt, gsl])
                            gt = tpool.tile([P, CHS], BF16, tag="tmp", name="gt")
                            nc.gpsimd.tensor_mul(out=gt[:],
                                                 in0=g_sb[:, gmt, gsl], in1=gd[:])
                            nc.gpsimd.tensor_add(out=g_sb[:, gmt, gsl],
                                                 in0=gt[:], in1=txtp2[:, gmt, gsl])
                gelu_scale = 1.0 if first else float(SA / SB)
                ev_scale = (1.0 / SA) if first else (1.0 / SB)
                # hidden [128, 16, 1024] fp8 x2 in the freed v0/v1 slots
                h_a = acts.tile([P, KT_F // 2, R], FP8, tag="S5",
                                name=f"h_a{pos}")
                h_b = acts.tile([P, KT_F // 2, R], FP8, tag="S6",
                                name=f"h_b{pos}")
                for mt in range(KT_F):
                    if first and mt < 2:
                        wt = w1_pre[mt]
                    else:
                        wt = load_w8(w1d, KT_H, mt, "w1")
                    hdst = h_a if mt < KT_F // 2 else h_b
                    for ch in range(CH):
                        ps = pmain.tile([P, CHS], F32, tag="mm", name="ps_f1")
                        mm_dr(ps, wt, r_f8, KT_H, ch)
                        nc.scalar.activation(
                            hdst[:, mt % (KT_F // 2), _chsl(ch)], ps[:],
                            AF.Gelu, bias=fb1_sb[:, mt:mt + 1], scale=gelu_scale)
                if first:
                    # reload the P8 operands here: the 4MB DMA would starve
                    # the w1 weight stream at the P67 head if issued earlier
                    nc.sync.dma_start(imgp2[:], imgp_d[:, :, :])
                    nc.sync.dma_start(txtp2[:], txtp_d[:, :, :])
                for mt in range(KT_H):
                    wt = load_w8(w2d, KT_F, mt, "w2")
                    for ch in range(CH):
                        ps = pmain.tile([P, CHS], F32, tag="mm", name="ps_f2")
                        for k in range(KT_F // 4):
                            nc.tensor.matmul(
                                ps[:], lhsT=wt[:, 2 * k:2 * k + 2, :],
                                rhs=h_a[:, 2 * k:2 * k + 2, _chsl(ch)],
                                start=(k == 0), stop=False, perf_mode=DR)
                        for k in range(KT_F // 4):
                            nc.tensor.matmul(
                                ps[:],
                                lhsT=wt[:, KT_F // 2 + 2 * k:KT_F // 2 + 2 * k + 2, :],
                                rhs=h_b[:, 2 * k:2 * k + 2, _chsl(ch)],
                                start=False, stop=(k == KT_F // 4 - 1),
                                perf_mode=DR)
                        if first:
                            nc.vector.scalar_tensor_tensor(
                                pooled[:, mt, _chsl(ch)], ps[:], ev_scale,
                                r_p[:, mt, _chsl(ch)], op0=ALU.mult, op1=ALU.add)
                        else:
                            # fused final combine: out = 0.5*(pooled0 + p1)
                            #   + g_sb (pre-combined gate term), streamed out
                            tmp = tpool.tile([P, CHS], F32, tag="tmp", name="ffn_tmp")
                            nc.vector.scalar_tensor_tensor(
                                tmp[:], ps[:], ev_scale,
                                r_p[:, mt, _chsl(ch)], op0=ALU.mult, op1=ALU.add)
                            nc.vector.tensor_add(out=tmp[:], in0=tmp[:],
                                                 in1=pooled[:, mt, _chsl(ch)])
                            fin = tpool.tile([P, CHS], F32, tag="tmp", name="gfin")
                            nc.vector.scalar_tensor_tensor(
                                fin[:], tmp[:], 0.5, g_sb[:, mt, _chsl(ch)],
                                op0=ALU.mult, op1=ALU.add)
                            nc.sync.dma_start(outT[mt * P:(mt + 1) * P, _chsl(ch)],
                                              fin[:])

        lnp_cm.__exit__(None, None, None)
        acts_cm.__exit__(None, None, None)
        spool_cm.__exit__(None, None, None)
        tpool_cm.__exit__(None, None, None)
        wpool_cm.__exit__(None, None, None)
        const_cm.__exit__(None, None, None)

    nc.compile()
    return nc


def host_prep(inputs):
    """Host-side preprocessing: merge CA weights, transpose, cast, shard."""
    f = {k: np.asarray(v, dtype=np.float32) for k, v in inputs.items()}

    def bf(x):
        return np.ascontiguousarray(x).astype(np_bf16)

    def q8(x, s):
        return np.ascontiguousarray(np.asarray(x, np.float32) * s).astype(np_fp8)

    def bias128(x, kt):
        return np.ascontiguousarray(np.asarray(x, np.float32).reshape(kt, P).T)

    ca_wv = np.split(f["ca_wqkv"], 3, axis=0)[2]
    ca_bv = f["ca_bqkv"][2 * H:]
    w_vo = f["ca_wo"] @ ca_wv
    b_vo = f["ca_wo"] @ ca_bv + f["ca_bo"]

    sa_wq, sa_wk, sa_wv = np.split(f["sa_wqkv"], 3, axis=0)
    sa_bq, sa_bk, sa_bv = np.split(f["sa_bqkv"], 3)

    gwi = f["gate_w"][:, :H]
    gwt = f["gate_w"][:, H:]

    lh = np.arange(P) // HD  # local head index within a 128-feature tile
    hmask = np.ascontiguousarray((lh[:, None] == np.arange(2)[None, :]).astype(np_bf16))
    hmaskT = np.ascontiguousarray(hmask.T)

    shared = {
        "wiT": bf(f["Wi"].T), "wtT": bf(f["Wt"].T),
        "wvoT": bf(w_vo.T),
        "wqT": q8(sa_wq.T, SA), "wqbT": q8(sa_wq.T, SB),
        "wkT": q8(sa_wk.T, SA),
        "wvT": q8(sa_wv.T, SA), "wvbT": q8(sa_wv.T, SB),
        "woT": q8(f["sa_wo"].T, SA),
        "w1aT": q8(f["ffn_w1"].T, SA), "w1bT": q8(f["ffn_w1"].T, SB),
        "w2aT": q8(f["ffn_w2"].T, SA), "w2bT": q8(f["ffn_w2"].T, SB),
        "gwiT": bf(gwi.T), "gwtT": bf(gwt.T),
        "bias_all": np.concatenate([
            bias128(f["bi"], KT_H), bias128(f["bt"], KT_H), bias128(b_vo, KT_H),
            bias128(sa_bq, KT_H), bias128(sa_bv / 8.0, KT_H),
            bias128(f["sa_bo"], KT_H), bias128(f["ffn_b2"], KT_H),
            bias128(f["gate_b"], KT_H),
            bias128(f["n1_g"], KT_H), bias128(f["n1_b"], KT_H),
            bias128(f["n2_g"], KT_H), bias128(f["n2_b"], KT_H),
            bias128(f["n3_g"], KT_H), bias128(f["n3_b"] + f["ffn_b2"], KT_H),
            bias128(f["n1_g"] / 8.0, KT_H), bias128(f["n1_b"] / 8.0, KT_H),
            bias128(f["n2_g"] / 8.0, KT_H), bias128(f["n2_b"] / 8.0, KT_H),
            bias128(f["n3_g"] / 8.0, KT_H), bias128(f["n3_b"] / 8.0, KT_H),
            bias128(f["ffn_b1"], KT_F),
        ], axis=1),
        "hmask": np.ascontiguousarray(hmask), "hmaskT": hmaskT,
    }

    xiT = f["image_features"].T.astype(np_bf16)  # [IMG_D, B]
    xtT = f["text_features"].T.astype(np_bf16)
    in_maps = []
    for c in range(N_CORES):
        m = dict(shared)
        m["xiT"] = np.ascontiguousarray(xiT[:, c * R:(c + 1) * R])
        m["xtT"] = np.ascontiguousarray(xtT[:, c * R:(c + 1) * R])
        in_maps.append(m)
    return in_maps


_NC_CACHE = None


def kernel(**inputs) -> np.ndarray:
    global _NC_CACHE
    if _NC_CACHE is None:
        _NC_CACHE = build()
    nc = _NC_CACHE
    in_maps = host_prep(inputs)
    res = run_bass_kernel_spmd(nc, in_maps, core_ids=list(range(N_CORES)))
    out = np.empty((B, H), np.float32)
    for c in range(N_CORES):
        out[c * R:(c + 1) * R, :] = res.results[c]["outT"].T
    return out


if __name__ == "__main__":
    nc = build()
    print("built OK")


# revision 22
# speedup vs baseline: 1.0012x; 1.0012x over previous
"""Trainium2 Bass kernel for nn_AttentionFusion (dense transformer block).

Data-parallel over batch: B=8192 rows sharded as 1024 rows per NeuronCore
across 8 cores; weights replicated. On-chip layout is feature-major:
activations are stored as [128 partitions(features), k_tiles, 1024 rows],
so every matmul is out.T[m,n] = sum_k W.T[k,m] * act.T[k,n] with natural
(host-pre-transposed) weight loads and the contraction on the partition dim.

Algebraic simplifications (validated against the reference to 2e-6):
  - Cross-attention has seq len 1 -> softmax == 1 -> out = v @ wo.T + bo;
    additionally (v @ wv.T) @ wo.T = v @ (wo@wv).T is merged on the host.
  - Self-attention has seq len 2 -> softmax([a,b]) = [sig(a-b), 1-sig(a-b)].
  - LayerNorm / attention-score reductions over features (= partitions) are
    done with small matmuls against ones/head-mask matrices.

fp8 (e4m3) DoubleRow matmuls (2x PE throughput, validated vs numpy model):
  - SA q/k/v, SA out-proj, FFN w1/w2 run as fp8 DoubleRow (contract 256/instr).
  - Weights are host-quantized at 8x scale (12x for the second FFN position to
    decorrelate quantization noise between the two pooled positions);
    activations are stored as value/8 in fp8, so PSUM results come out at true
    scale and evictions keep their 1-op baseline form.
  - Cross-attention / gate / input projections stay bf16 (noise budget).
"""

import numpy as np
import ml_dtypes

import concourse.bacc as bacc
import concourse.mybir as mybir
import concourse.tile as tile
from concourse.bass_utils import run_bass_kernel_spmd

AF = mybir.ActivationFunctionType
ALU = mybir.AluOpType
BF16 = mybir.dt.bfloat16
F32 = mybir.dt.float32
FP8 = mybir.dt.float8e4
DR = mybir.MatmulPerfMode.DoubleRow

N_CORES = 8
B, IMG_D, TXT_D, H, NH = 8192, 1280, 2048, 1024, 16
HD = H // NH  # 64 head dim
R = B // N_CORES  # 1024 rows per core
P = 128
CH = 2  # row chunks per core
CHS = R // CH  # 512 rows per chunk
KT_I, KT_T, KT_H, KT_F = IMG_D // P, TXT_D // P, H // P, 4 * H // P
EPS = 1e-5
SA = 8.0    # fp8 weight scale, position 0 / shared
SB = 12.0   # fp8 weight scale, position 1 (FFN dual-quantization)

np_bf16 = ml_dtypes.bfloat16
np_fp8 = ml_dtypes.float8_e4m3


def _chsl(ch):
    return slice(ch * CHS, (ch + 1) * CHS)


def build():
    nc = bacc.Bacc(None, target_bir_lowering=False)

    def din(name, shape, dt=BF16):
        return nc.dram_tensor(name, shape, dt, kind="ExternalInput")

    xiT = din("xiT", [IMG_D, R])
    xtT = din("xtT", [TXT_D, R])
    wiT = din("wiT", [IMG_D, H])
    wtT = din("wtT", [TXT_D, H])
    wvoT = din("wvoT", [H, H])
    wqT = din("wqT", [H, H], FP8)
    wqbT = din("wqbT", [H, H], FP8)
    wkT = din("wkT", [H, H], FP8)
    wvT = din("wvT", [H, H], FP8)
    wvbT = din("wvbT", [H, H], FP8)
    woT = din("woT", [H, H], FP8)
    w1aT = din("w1aT", [H, 4 * H], FP8)
    w1bT = din("w1bT", [H, 4 * H], FP8)
    w2aT = din("w2aT", [4 * H, H], FP8)
    w2bT = din("w2bT", [4 * H, H], FP8)
    gwiT = din("gwiT", [H, H])
    gwtT = din("gwtT", [H, H])

    bias_names = ["bi", "bt", "bvo", "sbq", "sbv8", "sbo", "fb2", "gb",
                  "n1g", "n1b", "n2g", "n2b", "n3gf", "n3bf",
                  "n1g8", "n1b8", "n2g8", "n2b8", "n3g8", "n3b8"]
    NB = len(bias_names) * KT_H + KT_F
    bias_all_d = din("bias_all", [P, NB], F32)
    hmask_d = din("hmask", [P, 2])
    hmaskT_d = din("hmaskT", [2, P])

    # DRAM spill for imgp/txtp between P3 and P8 (frees SBUF during SA/FFN)
    imgp_d = nc.dram_tensor("imgp_spill", [P, KT_H, R], BF16)
    txtp_d = nc.dram_tensor("txtp_spill", [P, KT_H, R], BF16)

    outT = nc.dram_tensor("outT", [H, R], F32, kind="ExternalOutput")

    with tile.TileContext(nc) as tc:
        def open_pool(**kw):
            cm = tc.tile_pool(**kw)
            return cm, cm.__enter__()

        def scope(name):
            import contextlib

            @contextlib.contextmanager
            def _s():
                sid, _ = nc.enter_named_scope(name, False)
                yield
                nc.leave_named_scope(name, sid, False)
            return _s()

        # -------- constants (whole kernel) --------
        const_cm, const = open_pool(name="const", bufs=1)
        ones128 = const.tile([P, P], BF16)
        nc.vector.memset(ones128[:], 1.0)
        eps_col = const.tile([P, 1], F32)
        nc.vector.memset(eps_col[:], EPS)
        zero_col = const.tile([P, 1], F32)
        nc.vector.memset(zero_col[:], 0.0)
        bias_all = const.tile([P, NB], F32)
        bias_sb = {n: bias_all[:, i * KT_H:(i + 1) * KT_H]
                   for i, n in enumerate(bias_names)}
        fb1_sb = bias_all[:, len(bias_names) * KT_H:]
        hmask_sb = const.tile([P, 2], BF16)
        hmaskT_sb = const.tile([2, P], BF16)

        def load_consts():
            nc.sync.dma_start(bias_all[:], bias_all_d[:, :])
            nc.sync.dma_start(hmask_sb[:], hmask_d[:, :])
            nc.sync.dma_start(hmaskT_sb[:], hmaskT_d[:, :])

        # -------- shared SBUF pools (whole kernel) --------
        wpool_cm, wpool = open_pool(name="wpool", bufs=2)   # "w" 4KB slots x2
        tpool_cm, tpool = open_pool(name="tpool", bufs=6)   # "tmp" 2KB x6
        spool_cm, spool = open_pool(name="spool", bufs=4)   # "small" 2KB x4
        acts_cm, acts = open_pool(name="acts", bufs=1)

        def act_tile(tag, name, dt=BF16, pad16=True):
            shape = [P, KT_H, R]
            pad = None
            if dt == FP8 and pad16:
                pad = [P, KT_H, 2 * R]  # keep the recycled slot at 16KB
            return acts.tile(shape, dt, tag=tag, name=name, padded_shape=pad)

        def sp_tile(name, shape=None, dt=BF16):
            """SP slot is 16KB/partition (bf16 [P, KT_H, R])."""
            shape = shape or [P, KT_H, R]
            pad = None
            if mybir.dt.size(dt) == 1:
                pad = [shape[0], shape[1], shape[2] * 2]
            return acts.tile(shape, dt, tag="SP", name=name, padded_shape=pad)

        pmain = None
        paux = None

        def load_w(wT_d, kt, mt, name):
            """bf16 weight slice wT[:, mt*128:(mt+1)*128] as [128, kt, 128]."""
            if kt > KT_H:
                t = wpool.tile([P, KT_F, P], BF16, tag="w", name=name)
            else:
                t = wpool.tile([P, KT_H, P], BF16, tag="w_h", name=name, bufs=6)
            nc.sync.dma_start(
                t[:, :kt, :],
                wT_d[:, mt * P:(mt + 1) * P].rearrange("(k p) m -> p k m", p=P),
            )
            return t

        def load_w8(wT_d, kt, mt, name):
            """fp8 weight slice as [128, kt, 128] (big 'w' slot is 4KB fp8)."""
            if kt > KT_H:
                t = wpool.tile([P, KT_F, P], FP8, tag="w", name=name)
            else:
                t = wpool.tile([P, KT_H, P], FP8, tag="w_h", name=name, bufs=6,
                               padded_shape=[P, KT_H, 2 * P])
            nc.sync.dma_start(
                t[:, :kt, :],
                wT_d[:, mt * P:(mt + 1) * P].rearrange("(k p) m -> p k m", p=P),
            )
            return t

        def mm_layer(srcs, mt, evict, wname="w", chs=range(CH)):
            wts = [load_w(w_d, kt, mt, f"{wname}{i}") for i, (w_d, _, kt) in enumerate(srcs)]
            nk_tot = sum(kt for (_, _, kt) in srcs)
            for ch in chs:
                ps = pmain.tile([P, CHS], F32, tag="mm", name="ps_mm")
                i = 0
                for (w_d, act, kt), wt in zip(srcs, wts):
                    for k in range(kt):
                        nc.tensor.matmul(
                            ps[:], lhsT=wt[:, k, :], rhs=act[:, k, _chsl(ch)],
                            start=(i == 0), stop=(i == nk_tot - 1),
                        )
                        i += 1
                evict(mt, ch, ps)

        def mm_dr(ps, wt, act8, kt, ch, start=True, stop=True):
            """fp8 DoubleRow accumulation chain: kt k-tiles as kt//2 pairs."""
            np_ = kt // 2
            for k in range(np_):
                nc.tensor.matmul(
                    ps[:], lhsT=wt[:, 2 * k:2 * k + 2, :],
                    rhs=act8[:, 2 * k:2 * k + 2, _chsl(ch)],
                    start=(start and k == 0), stop=(stop and k == np_ - 1),
                    perf_mode=DR,
                )

        def evict_bias(dst, bname):
            b = bias_sb[bname]

            def _e(mt, ch, ps):
                nc.scalar.activation(
                    dst[:, mt, _chsl(ch)], ps[:], AF.Identity,
                    bias=b[:, mt:mt + 1], scale=1.0,
                )
            return _e

        def evict_bias_res(dst, bname, res):
            b = bias_sb[bname]

            def _e(mt, ch, ps):
                nc.vector.scalar_tensor_tensor(
                    dst[:, mt, _chsl(ch)], ps[:], b[:, mt:mt + 1],
                    res[:, mt, _chsl(ch)], op0=ALU.add, op1=ALU.add,
                )
            return _e

        lnp_cm, lnp = open_pool(name="lnp", bufs=4)  # LN stats (mf bf16, ivf f32)

        def ln_stats(x_bf, ch):
            """Row mean + rsqrt(var) via ones-matmuls. Emitted for ALL chunks
            before any normalize pass so the x^2 ACT feeds are not queued
            behind the normalize evict ACTs (which stalled the Qb matmuls)."""
            sb_ = paux.tile([P, CHS], F32, tag="Sb", name="ln_Sb")
            for k in range(KT_H):
                nc.tensor.matmul(sb_[:], lhsT=ones128[:],
                                 rhs=x_bf[:, k, _chsl(ch)],
                                 start=(k == 0), stop=(k == KT_H - 1))
            qb_ = paux.tile([P, CHS], F32, tag="Qb", name="ln_Qb")
            for k in range(KT_H):
                x2 = tpool.tile([P, CHS], BF16, tag="tmp", name="ln_x2")
                nc.vector.tensor_mul(out=x2[:], in0=x_bf[:, k, _chsl(ch)],
                                     in1=x_bf[:, k, _chsl(ch)])
                nc.tensor.matmul(qb_[:], lhsT=ones128[:], rhs=x2[:],
                                 start=(k == 0), stop=(k == KT_H - 1))
            mf = lnp.tile([P, CHS], BF16, tag="lnm", name="ln_mf")
            nc.vector.tensor_scalar_mul(mf[:], sb_[:], 1.0 / H)
            msq = tpool.tile([P, CHS], F32, tag="tmp", name="ln_msq")
            nc.vector.tensor_mul(out=msq[:], in0=mf[:], in1=mf[:])
            vf = tpool.tile([P, CHS], F32, tag="tmp", name="ln_vf")
            nc.vector.scalar_tensor_tensor(vf[:], qb_[:], 1.0 / H, msq[:],
                                           op0=ALU.mult, op1=ALU.subtract)
            sd = tpool.tile([P, CHS], F32, tag="tmp", name="ln_sd")
            nc.scalar.activation(sd[:], vf[:], AF.Sqrt, bias=eps_col[:], scale=1.0)
            # ~5x faster than nc.vector.reciprocal (which stalled PE 3.4us)
            ivf_f = tpool.tile([P, CHS], F32, tag="tmp", name="ln_ivf_f")
            nc.vector.reciprocal_approx_fast(out=ivf_f[:], in_=sd[:])
            ivf = lnp.tile([P, CHS], BF16, tag="lni", name="ln_ivf")
            nc.vector.tensor_scalar_mul(ivf[:], ivf_f[:], 1.0)
            return mf, ivf

        def ln_norm(x_bf, stats, ch, g_name, b_name, out_bf,
                    out_f8=None, g8_name=None, b8_name=None, f8_eng=None):
            g = bias_sb[g_name]
            bb = bias_sb[b_name]
            mf, ivf = stats
            for k in range(KT_H):
                t1 = tpool.tile([P, CHS], BF16, tag="tmp", name="ln_t1")
                nc.vector.tensor_sub(out=t1[:], in0=x_bf[:, k, _chsl(ch)], in1=mf[:])
                t2 = tpool.tile([P, CHS], BF16, tag="tmp", name="ln_t2")
                nc.vector.tensor_mul(out=t2[:], in0=t1[:], in1=ivf[:])
                nc.vector.tensor_scalar(out_bf[:, k, _chsl(ch)], t2[:],
                                        g[:, k:k + 1], bb[:, k:k + 1],
                                        op0=ALU.mult, op1=ALU.add)
                if out_f8 is not None:
                    g8 = bias_sb[g8_name]
                    b8 = bias_sb[b8_name]
                    (f8_eng or nc.vector).tensor_scalar(
                        out_f8[:, k, _chsl(ch)], t2[:],
                        g8[:, k:k + 1], b8[:, k:k + 1],
                        op0=ALU.mult, op1=ALU.add)

        def layernorm(x_bf, g_name, b_name, out_bf,
                      out_f8=None, g8_name=None, b8_name=None):
            st = [ln_stats(x_bf, ch) for ch in range(CH)]
            for ch in range(CH):
                ln_norm(x_bf, st[ch], ch, g_name, b_name, out_bf,
                        out_f8, g8_name, b8_name)

        # ================= P0/P1: input projections (streamed) =============
        imgp = act_tile("S1", "imgp")
        txtp = act_tile("S2", "txtp")

        def input_proj(xT_d, w_d, kt_in, bname, dst, post_dma=None):
            for ch in range(CH):
                pss = [pmain.tile([P, CHS], F32, tag=f"mm{mt}", name=f"ps{mt}")
                       for mt in range(KT_H)]
                for k in range(kt_in):
                    wt = wpool.tile([P, H], BF16, tag="w_h", name="wrow", bufs=6)
                    nc.sync.dma_start(wt[:], w_d[k * P:(k + 1) * P, :])
                    xs = tpool.tile([P, CHS], BF16, tag="tmp", name="xslice")
                    nc.sync.dma_start(xs[:], xT_d[k * P:(k + 1) * P, _chsl(ch)])
                    for mt in range(KT_H):
                        nc.tensor.matmul(pss[mt][:], lhsT=wt[:, mt * P:(mt + 1) * P],
                                         rhs=xs[:], start=(k == 0), stop=(k == kt_in - 1))
                if post_dma is not None:
                    post_dma()
                    post_dma = None
                for mt in range(KT_H):
                    nc.scalar.activation(dst[:, mt, _chsl(ch)], pss[mt][:], AF.Identity,
                                         bias=bias_sb[bname][:, mt:mt + 1], scale=1.0)

        with scope("P01"), tc.tile_pool(name="pmm01", bufs=1, space="PSUM") as pmain:
            input_proj(xiT, wiT, KT_I, "bi", imgp, post_dma=load_consts)
            input_proj(xtT, wtT, KT_T, "bt", txtp)
            nc.sync.dma_start(imgp_d[:, :, :], imgp[:])
            nc.sync.dma_start(txtp_d[:, :, :], txtp[:])
            # prefetch P23's first two weight tiles into the big-weight slots
            # (sized to match the 4KB fp8 FFN slices that share the tag)
            wvo_pre = []
            for mt in range(2):
                t = wpool.tile([P, KT_H, P], BF16, tag="w", name=f"wvo_pre{mt}",
                               padded_shape=[P, 2 * KT_H, P])
                nc.sync.dma_start(
                    t[:, :, :],
                    wvoT[:, mt * P:(mt + 1) * P].rearrange("(k p) m -> p k m", p=P))
                wvo_pre.append(t)

        # ============ P2/P3: merged cross-attention + LN ============
        c0 = act_tile("S3", "c0")
        c1 = act_tile("S4", "c1")
        c0_f8 = act_tile("C8a", "c0_f8", FP8, pad16=False)  # c0/8 for DR rhs
        c1_f8 = act_tile("C8b", "c1_f8", FP8, pad16=False)

        with (
            scope("P23"),
            tc.tile_pool(name="pmm23", bufs=4, space="PSUM") as pmain,
            tc.tile_pool(name="paux23", bufs=2, space="PSUM") as paux,
            tc.tile_pool(name="pca", bufs=1) as pca,
        ):
            # GEMMs first (x0 then x1, weights loaded once per mt); both LN
            # chains afterwards so their ACT/DVE tails hide under the gate
            # GEMMs of P3g (PE never waits on LN).
            x0 = sp_tile("x0")
            ev_x0 = evict_bias_res(x0, "bvo", imgp)
            for mt in range(KT_H):
                if mt < 2:
                    for ch in range(CH):
                        ps = pmain.tile([P, CHS], F32, tag="mm", name="ps_mm")
                        for k in range(KT_H):
                            nc.tensor.matmul(ps[:], lhsT=wvo_pre[mt][:, k, :],
                                             rhs=txtp[:, k, _chsl(ch)],
                                             start=(k == 0), stop=(k == KT_H - 1))
                        ev_x0(mt, ch, ps)
                else:
                    mm_layer([(wvoT, txtp, KT_H)], mt, ev_x0, wname="wvo")
            x1 = pca.tile([P, KT_H, R], BF16, tag="x1", name="x1")
            for mt in range(KT_H):
                mm_layer([(wvoT, imgp, KT_H)], mt, evict_bias_res(x1, "bvo", txtp),
                         wname="wvo")
            st0 = [ln_stats(x0, ch) for ch in range(CH)]
            st1 = [ln_stats(x1, ch) for ch in range(CH)]
            for ch in range(CH):
                ln_norm(x0, st0[ch], ch, "n1g", "n1b", c0,
                        c0_f8, "n1g8", "n1b8", f8_eng=nc.gpsimd)
            cd_f8 = sp_tile("cd", dt=FP8)  # (c0-c1)/8, built per tile
            for ch in range(CH):
                ln_norm(x1, st1[ch], ch, "n2g", "n2b", c1,
                        c1_f8, "n2g8", "n2b8", f8_eng=nc.gpsimd)
                for k in range(KT_H):
                    nc.gpsimd.tensor_sub(out=cd_f8[:, k, _chsl(ch)],
                                         in0=c0_f8[:, k, _chsl(ch)],
                                         in1=c1_f8[:, k, _chsl(ch)])

        # ================ P3g: gate logits (independent filler) ==========
        g_sb = act_tile("SG", "g_sb")
        with (
            scope("P3g"),
            tc.tile_pool(name="pmm3g", bufs=8, space="PSUM") as pmain,
        ):
            for mt in range(KT_H):
                wgi = load_w(gwiT, KT_H, mt, "wgi")
                wgt = load_w(gwtT, KT_H, mt, "wgt")
                for ch in range(CH):
                    ps = pmain.tile([P, CHS], F32, tag="mm", name="ps_g")
                    for k in range(KT_H):
                        nc.tensor.matmul(ps[:], lhsT=wgi[:, k, :],
                                         rhs=imgp[:, k, _chsl(ch)],
                                         start=(k == 0), stop=False)
                    for k in range(KT_H):
                        nc.tensor.matmul(ps[:], lhsT=wgt[:, k, :],
                                         rhs=txtp[:, k, _chsl(ch)],
                                         start=False, stop=(k == KT_H - 1))
                    if (mt + ch) % 2 == 0:
                        nc.scalar.activation(g_sb[:, mt, _chsl(ch)], ps[:], AF.Identity,
                                             bias=bias_sb["gb"][:, mt:mt + 1], scale=1.0)
                    else:
                        nc.vector.tensor_scalar(g_sb[:, mt, _chsl(ch)], ps[:],
                                                bias_sb["gb"][:, mt:mt + 1], None,
                                                op0=ALU.add)

        # ============ P4: self-attention qkv + scores (fp8 DR) ============
        v0 = act_tile("S5", "v0")   # stored as v/8 (bf16)
        v1 = act_tile("S6", "v1")
        o0 = act_tile("S1", "o0", FP8)  # o/8 in fp8, after imgp's last read
        o1 = act_tile("S2", "o1", FP8)

        with (
            scope("P4"),
            tc.tile_pool(name="pmm4", bufs=3, space="PSUM") as pmain,
            tc.tile_pool(name="pd", bufs=2, space="PSUM") as pd,
            tc.tile_pool(name="pab", bufs=1, space="PSUM") as pab,
            tc.tile_pool(name="pqk", bufs=1) as pqk,
        ):
            def qkv8(wt, act8, bname, mt, dst_t, dst_mt=None, scale=1.0):
                for ch in range(CH):
                    ps = pmain.tile([P, CHS], F32, tag="mm", name="ps_qkv")
                    mm_dr(ps, wt, act8, KT_H, ch)
                    bias = bias_sb[bname][:, mt:mt + 1] if bname else zero_col[:, :]
                    if dst_mt is None:
                        nc.scalar.activation(dst_t[:, _chsl(ch)], ps[:], AF.Identity,
                                             bias=bias, scale=scale)
                    else:
                        nc.scalar.activation(dst_t[:, dst_mt, _chsl(ch)], ps[:],
                                             AF.Identity, bias=bias, scale=scale)

            hm2 = hmask_sb[:, :]    # [128, 2] local-head one-hot
            hmT2 = hmaskT_sb[:, :]  # [2, 128]
            AB = float(SA / SB)
            for mt in range(KT_H):
                wq_t = load_w8(wqT, KT_H, mt, "wq")
                wv_t = load_w8(wvT, KT_H, mt, "wv")
                wk_t = load_w8(wkT, KT_H, mt, "wk")
                wqb_t = load_w8(wqbT, KT_H, mt, "wqb")
                wvb_t = load_w8(wvbT, KT_H, mt, "wvb")
                q0t = pqk.tile([P, R], BF16, tag="q0t")
                q1t = pqk.tile([P, R], BF16, tag="q1t")
                kdt = pqk.tile([P, R], BF16, tag="kdt")
                qkv8(wq_t, c0_f8, "sbq", mt, q0t)
                qkv8(wv_t, c0_f8, "sbv8", mt, v0, dst_mt=mt, scale=1.0 / 8.0)
                qkv8(wqb_t, c1_f8, "sbq", mt, q1t, scale=AB)
                qkv8(wk_t, cd_f8, None, mt, kdt)  # k0-k1; bias cancels
                nc.vector.tensor_mul(out=q0t[:], in0=q0t[:], in1=kdt[:])
                nc.vector.tensor_mul(out=q1t[:], in0=q1t[:], in1=kdt[:])
                m0, m1 = q0t, q1t
                a_ts = {}
                for ch in range(CH):
                    for m_t, nm in ((m0, "A"), (m1, "B")):
                        dmm = pd.tile([2, CHS], F32, tag="dmm", name=f"dmm{nm}")
                        nc.tensor.matmul(dmm[:], lhsT=hm2, rhs=m_t[:, _chsl(ch)],
                                         start=True, stop=True)
                        a_t = spool.tile([2, CHS], BF16, tag="small", name=f"a{nm}")
                        nc.scalar.activation(a_t[:], dmm[:], AF.Sigmoid,
                                             bias=zero_col[0:2, :],
                                             scale=float(1.0 / np.sqrt(HD)))
                        a_ts[(ch, nm)] = a_t
                qkv8(wvb_t, c1_f8, "sbv8", mt, v1, dst_mt=mt, scale=1.0 / SB)
                for ch in range(CH):
                    diff = tpool.tile([P, CHS], BF16, tag="tmp", name="att_diff")
                    nc.vector.tensor_sub(out=diff[:], in0=v0[:, mt, _chsl(ch)],
                                         in1=v1[:, mt, _chsl(ch)])
                    for o_t, nm in ((o0, "A"), (o1, "B")):
                        ab = pab.tile([P, CHS], F32, tag=f"ab{nm}", name=f"ab{nm}")
                        nc.tensor.matmul(ab[:], lhsT=hmT2, rhs=a_ts[(ch, nm)][:],
                                         start=True, stop=True)
                        t_t = tpool.tile([P, CHS], BF16, tag="tmp", name=f"att_t{nm}")
                        nc.vector.tensor_mul(out=t_t[:], in0=diff[:], in1=ab[:])
                        nc.vector.tensor_add(out=o_t[:, mt, _chsl(ch)], in0=t_t[:],
                                             in1=v1[:, mt, _chsl(ch)])

        # prefetch the first two FFN w1 tiles into the free big-weight slots
        # so P67 starts without a DMA wait
        w1_pre = []
        for mt in range(2):
            t = wpool.tile([P, KT_F, P], FP8, tag="w", name=f"w1_pre{mt}")
            nc.sync.dma_start(
                t[:, :KT_H, :],
                w1aT[:, mt * P:(mt + 1) * P].rearrange("(k p) m -> p k m", p=P))
            w1_pre.append(t)

        # ===== P5: SA out-proj (fp8 DR) + residual + LN3 =====
        # LN3's bf16 output is stored as r + ffn_b2 (bias n3bf = n3_b + fb2) so
        # the FFN w2 evict needs no extra bias op; the fp8 copy holds r/8.
        r0 = act_tile("S1", "r0")    # r0 + fb2 (bf16); reuses o0 slot
        r1 = act_tile("S2", "r1")
        r0_f8 = act_tile("C8a", "r0_f8", FP8, pad16=False)  # reuse c_f8 slots
        r1_f8 = act_tile("C8b", "r1_f8", FP8, pad16=False)
        with (
            scope("P5"),
            tc.tile_pool(name="pmm5", bufs=6, space="PSUM") as pmain,
            tc.tile_pool(name="paux5", bufs=1, space="PSUM") as paux,
            tc.tile_pool(name="psa", bufs=1) as psa,
        ):
            y0 = sp_tile("y0")
            y1 = psa.tile([P, KT_H, R], BF16, tag="y1", name="y1")

            # wo loaded once per mt, both positions; LN3 chains emitted after
            # all GEMMs so their ACT/DVE tails hide under LN3-y1 / P67 PE work.
            for mt in range(KT_H):
                wt = load_w8(woT, KT_H, mt, "wo")
                for o_t, res, dst in ((o0, c0, y0), (o1, c1, y1)):
                    for ch in range(CH):
                        ps = pmain.tile([P, CHS], F32, tag="mm", name="ps_wo")
                        mm_dr(ps, wt, o_t, KT_H, ch)
                        nc.vector.scalar_tensor_tensor(
                            dst[:, mt, _chsl(ch)], ps[:],
                            bias_sb["sbo"][:, mt:mt + 1],
                            res[:, mt, _chsl(ch)], op0=ALU.add, op1=ALU.add)
            sty0 = [ln_stats(y0, ch) for ch in range(CH)]
            for ch in range(CH):
                ln_norm(y0, sty0[ch], ch, "n3gf", "n3bf", r0,
                        r0_f8, "n3g8", "n3b8")
            sty1 = [ln_stats(y1, ch) for ch in range(CH)]
            for ch in range(CH):
                ln_norm(y1, sty1[ch], ch, "n3gf", "n3bf", r1,
                        r1_f8, "n3g8", "n3b8")

        # gate sigmoid in place (ACT is idle here; shortens the P8 tail)
        for mt in range(KT_H):
            for ch in range(CH):
                nc.scalar.activation(g_sb[:, mt, _chsl(ch)], g_sb[:, mt, _chsl(ch)],
                                     AF.Sigmoid, bias=zero_col[:], scale=1.0)

        imgp2 = act_tile("S3", "imgp2")
        txtp2 = act_tile("S4", "txtp2")

        # ===== P6/P7: FFN both positions (fp8 DR); pooled accumulation =====
        # mt-outer / ch-inner: each weight tile is loaded once per position
        # (halves FFN weight DMA); hidden tiles hold the full row range.
        pooled = sp_tile("pooled")  # bf16; pos1 fuses the final combine
        with (
            scope("P67"),
            tc.tile_pool(name="pmm67", bufs=8, space="PSUM") as pmain,
        ):
            gate_done = False
            for pos, (r_f8, r_p, w1d, w2d, first) in enumerate([
                    (r0_f8, r0, w1aT, w2aT, True),
                    (r1_f8, r1, w1bT, w2bT, False)]):
                if not first and not gate_done:
                    # g_sb <- gate*(imgp-txtp) + txtp in place; runs on DVE
                    # during pos0's PE work so the pos1 chain is 3 ops + DMA
                    gate_done = True
                    for gmt in range(KT_H):
                        for gch in range(CH):
                            gsl = _chsl(gch)
                            gd = tpool.tile([P, CHS], BF16, tag="tmp", name="gd")
                            nc.gpsimd.tensor_sub(out=gd[:],
                                                 in0=imgp2[:, gmt, gsl],
                                                 in1=txtp2[:, gmt, gsl])
                            gt = tpool.tile([P, CHS], BF16, tag="tmp", name="gt")
                            nc.gpsimd.tensor_mul(out=gt[:],
                                                 in0=g_sb[:, gmt, gsl], in1=gd[:])
                            nc.gpsimd.tensor_add(out=g_sb[:, gmt, gsl],
                                                 in0=gt[:], in1=txtp2[:, gmt, gsl])
                gelu_scale = 1.0 if first else float(SA / SB)
                ev_scale = (1.0 / SA) if first else (1.0 / SB)
                # hidden [128, 16, 1024] fp8 x2 in the freed v0/v1 slots
                h_a = acts.tile([P, KT_F // 2, R], FP8, tag="S5",
                                name=f"h_a{pos}")
                h_b = acts.tile([P, KT_F // 2, R], FP8, tag="S6",
                                name=f"h_b{pos}")
                for mt in range(KT_F):
                    if first and mt < 2:
                        wt = w1_pre[mt]
                    else:
                        wt = load_w8(w1d, KT_H, mt, "w1")
                    hdst = h_a if mt < KT_F // 2 else h_b
                    for ch in range(CH):
                        ps = pmain.tile([P, CHS], F32, tag="mm", name="ps_f1")
                        mm_dr(ps, wt, r_f8, KT_H, ch)
                        nc.scalar.activation(
                            hdst[:, mt % (KT_F // 2), _chsl(ch)], ps[:],
                            AF.Gelu, bias=fb1_sb[:, mt:mt + 1], scale=gelu_scale)
                if first:
                    # reload the P8 operands here: the 4MB DMA would starve
                    # the w1 weight stream at the P67 head if issued earlier
                    nc.sync.dma_start(imgp2[:], imgp_d[:, :, :])
                    nc.sync.dma_start(txtp2[:], txtp_d[:, :, :])
                for mt in range(KT_H):
                    wt = load_w8(w2d, KT_F, mt, "w2")
                    for ch in range(CH):
                        ps = pmain.tile([P, CHS], F32, tag="mm", name="ps_f2")
                        for k in range(KT_F // 4):
                            nc.tensor.matmul(
                                ps[:], lhsT=wt[:, 2 * k:2 * k + 2, :],
                                rhs=h_a[:, 2 * k:2 * k + 2, _chsl(ch)],
                                start=(k == 0), stop=False, perf_mode=DR)
                        for k in range(KT_F // 4):
                            nc.tensor.matmul(
                                ps[:],
                                lhsT=wt[:, KT_F // 2 + 2 * k:KT_F // 2 + 2 * k + 2, :],
                                rhs=h_b[:, 2 * k:2 * k + 2, _chsl(ch)],
                                start=False, stop=(k == KT_F // 4 - 1),
                                perf_mode=DR)
                        if first:
                            nc.vector.scalar_tensor_tensor(
                                pooled[:, mt, _chsl(ch)], ps[:], ev_scale,
                                r_p[:, mt, _chsl(ch)], op0=ALU.mult, op1=ALU.add)
                        else:
                            # fused final combine: out = 0.5*(pooled0 + p1)
                            #   + g_sb (pre-combined gate term), streamed out
                            tmp = tpool.tile([P, CHS], F32, tag="tmp", name="ffn_tmp")
                            nc.vector.scalar_tensor_tensor(
                                tmp[:], ps[:], ev_scale,
                                r_p[:, mt, _chsl(ch)], op0=ALU.mult, op1=ALU.add)
                            nc.vector.tensor_add(out=tmp[:], in0=tmp[:],
                                                 in1=pooled[:, mt, _chsl(ch)])
                            fin = tpool.tile([P, CHS], F32, tag="tmp", name="gfin")
                            nc.vector.scalar_tensor_tensor(
                                fin[:], tmp[:], 0.5, g_sb[:, mt, _chsl(ch)],
                                op0=ALU.mult, op1=ALU.add)
                            nc.sync.dma_start(outT[mt * P:(mt + 1) * P, _chsl(ch)],
                                              fin[:])

        lnp_cm.__exit__(None, None, None)
        acts_cm.__exit__(None, None, None)
        spool_cm.__exit__(None, None, None)
        tpool_cm.__exit__(None, None, None)
        wpool_cm.__exit__(None, None, None)
        const_cm.__exit__(None, None, None)

    nc.compile()
    return nc


def host_prep(inputs):
    """Host-side preprocessing: merge CA weights, transpose, cast, shard."""
    f = {k: np.asarray(v, dtype=np.float32) for k, v in inputs.items()}

    def bf(x):
        return np.ascontiguousarray(x).astype(np_bf16)

    def q8(x, s):
        return np.ascontiguousarray(np.asarray(x, np.float32) * s).astype(np_fp8)

    def bias128(x, kt):
        return np.ascontiguousarray(np.asarray(x, np.float32).reshape(kt, P).T)

    ca_wv = np.split(f["ca_wqkv"], 3, axis=0)[2]
    ca_bv = f["ca_bqkv"][2 * H:]
    w_vo = f["ca_wo"] @ ca_wv
    b_vo = f["ca_wo"] @ ca_bv + f["ca_bo"]

    sa_wq, sa_wk, sa_wv = np.split(f["sa_wqkv"], 3, axis=0)
    sa_bq, sa_bk, sa_bv = np.split(f["sa_bqkv"], 3)

    gwi = f["gate_w"][:, :H]
    gwt = f["gate_w"][:, H:]

    lh = np.arange(P) // HD  # local head index within a 128-feature tile
    hmask = np.ascontiguousarray((lh[:, None] == np.arange(2)[None, :]).astype(np_bf16))
    hmaskT = np.ascontiguousarray(hmask.T)

    shared = {
        "wiT": bf(f["Wi"].T), "wtT": bf(f["Wt"].T),
        "wvoT": bf(w_vo.T),
        "wqT": q8(sa_wq.T, SA), "wqbT": q8(sa_wq.T, SB),
        "wkT": q8(sa_wk.T, SA),
        "wvT": q8(sa_wv.T, SA), "wvbT": q8(sa_wv.T, SB),
        "woT": q8(f["sa_wo"].T, SA),
        "w1aT": q8(f["ffn_w1"].T, SA), "w1bT": q8(f["ffn_w1"].T, SB),
        "w2aT": q8(f["ffn_w2"].T, SA), "w2bT": q8(f["ffn_w2"].T, SB),
        "gwiT": bf(gwi.T), "gwtT": bf(gwt.T),
        "bias_all": np.concatenate([
            bias128(f["bi"], KT_H), bias128(f["bt"], KT_H), bias128(b_vo, KT_H),
            bias128(sa_bq, KT_H), bias128(sa_bv / 8.0, KT_H),
            bias128(f["sa_bo"], KT_H), bias128(f["ffn_b2"], KT_H),
            bias128(f["gate_b"], KT_H),
            bias128(f["n1_g"], KT_H), bias128(f["n1_b"], KT_H),
            bias128(f["n2_g"], KT_H), bias128(f["n2_b"], KT_H),
            bias128(f["n3_g"], KT_H), bias128(f["n3_b"] + f["ffn_b2"], KT_H),
            bias128(f["n1_g"] / 8.0, KT_H), bias128(f["n1_b"] / 8.0, KT_H),
            bias128(f["n2_g"] / 8.0, KT_H), bias128(f["n2_b"] / 8.0, KT_H),
            bias128(f["n3_g"] / 8.0, KT_H), bias128(f["n3_b"] / 8.0, KT_H),
            bias128(f["ffn_b1"], KT_F),
        ], axis=1),
        "hmask": np.ascontiguousarray(hmask), "hmaskT": hmaskT,
    }

    xiT = f["image_features"].T.astype(np_bf16)  # [IMG_D, B]
    xtT = f["text_features"].T.astype(np_bf16)
    in_maps = []
    for c in range(N_CORES):
        m = dict(shared)
        m["xiT"] = np.ascontiguousarray(xiT[:, c * R:(c + 1) * R])
        m["xtT"] = np.ascontiguousarray(xtT[:, c * R:(c + 1) * R])
        in_maps.append(m)
    return in_maps


_NC_CACHE = None


def kernel(**inputs) -> np.ndarray:
    global _NC_CACHE
    if _NC_CACHE is None:
        _NC_CACHE = build()
    nc = _NC_CACHE
    in_maps = host_prep(inputs)
    res = run_bass_kernel_spmd(nc, in_maps, core_ids=list(range(N_CORES)))
    out = np.empty((B, H), np.float32)
    for c in range(N_CORES):
        out[c * R:(c + 1) * R, :] = res.results[c]["outT"].T
    return out


if __name__ == "__main__":
    nc = build()
    print("built OK")


# revision 23
# speedup vs baseline: 1.0341x; 1.0329x over previous
"""Trainium2 Bass kernel for nn_AttentionFusion (dense transformer block).

Data-parallel over batch: B=8192 rows sharded as 1024 rows per NeuronCore
across 8 cores; weights replicated. On-chip layout is feature-major:
activations are stored as [128 partitions(features), k_tiles, 1024 rows],
so every matmul is out.T[m,n] = sum_k W.T[k,m] * act.T[k,n] with natural
(host-pre-transposed) weight loads and the contraction on the partition dim.

Algebraic simplifications (validated against the reference to 2e-6):
  - Cross-attention has seq len 1 -> softmax == 1 -> out = v @ wo.T + bo;
    additionally (v @ wv.T) @ wo.T = v @ (wo@wv).T is merged on the host.
  - Self-attention has seq len 2 -> softmax([a,b]) = [sig(a-b), 1-sig(a-b)].
  - LayerNorm / attention-score reductions over features (= partitions) are
    done with small matmuls against ones/head-mask matrices.

fp8 (e4m3) DoubleRow matmuls (2x PE throughput, validated vs numpy model):
  - SA q/k/v, SA out-proj, FFN w1/w2 run as fp8 DoubleRow (contract 256/instr).
  - Weights are host-quantized at 8x scale (12x for the second FFN position to
    decorrelate quantization noise between the two pooled positions);
    activations are stored as value/8 in fp8, so PSUM results come out at true
    scale and evictions keep their 1-op baseline form.
  - Cross-attention / gate / input projections stay bf16 (noise budget).
"""

import numpy as np
import ml_dtypes

import concourse.bacc as bacc
import concourse.mybir as mybir
import concourse.tile as tile
from concourse.bass_utils import run_bass_kernel_spmd

AF = mybir.ActivationFunctionType
ALU = mybir.AluOpType
BF16 = mybir.dt.bfloat16
F32 = mybir.dt.float32
FP8 = mybir.dt.float8e4
DR = mybir.MatmulPerfMode.DoubleRow

N_CORES = 8
B, IMG_D, TXT_D, H, NH = 8192, 1280, 2048, 1024, 16
HD = H // NH  # 64 head dim
R = B // N_CORES  # 1024 rows per core
P = 128
CH = 2  # row chunks per core
CHS = R // CH  # 512 rows per chunk
KT_I, KT_T, KT_H, KT_F = IMG_D // P, TXT_D // P, H // P, 4 * H // P
EPS = 1e-5
SA = 8.0    # fp8 weight scale, position 0 / shared
SB = 12.0   # fp8 weight scale, position 1 (FFN dual-quantization)

np_bf16 = ml_dtypes.bfloat16
np_fp8 = ml_dtypes.float8_e4m3


def _chsl(ch):
    return slice(ch * CHS, (ch + 1) * CHS)


def build():
    nc = bacc.Bacc(None, target_bir_lowering=False)

    def din(name, shape, dt=BF16):
        return nc.dram_tensor(name, shape, dt, kind="ExternalInput")

    xiT = din("xiT", [IMG_D, R])
    xtT = din("xtT", [TXT_D, R])
    wiT = din("wiT", [IMG_D, H])
    wtT = din("wtT", [TXT_D, H])
    wvoT = din("wvoT", [H, H])
    wqT = din("wqT", [H, H], FP8)
    wqbT = din("wqbT", [H, H], FP8)
    wkT = din("wkT", [H, H], FP8)
    wvT = din("wvT", [H, H], FP8)
    wvbT = din("wvbT", [H, H], FP8)
    woT = din("woT", [H, H], FP8)
    w1aT = din("w1aT", [H, 4 * H], FP8)
    w1bT = din("w1bT", [H, 4 * H], FP8)
    w2aT = din("w2aT", [4 * H, H], FP8)
    w2bT = din("w2bT", [4 * H, H], FP8)
    gwiT = din("gwiT", [H, H])
    gwtT = din("gwtT", [H, H])

    bias_names = ["bi", "bt", "bvo", "sbq", "sbv8", "sbo", "fb2", "gb",
                  "n1g", "n1b", "n2g", "n2b", "n3gf", "n3bf",
                  "n1g8", "n1b8", "n2g8", "n2b8", "n3g8", "n3b8"]
    NB = len(bias_names) * KT_H + KT_F
    bias_all_d = din("bias_all", [P, NB], F32)
    hmask_d = din("hmask", [P, 2])
    hmaskT_d = din("hmaskT", [2, P])

    # DRAM spill for imgp/txtp between P3 and P8 (frees SBUF during SA/FFN)
    imgp_d = nc.dram_tensor("imgp_spill", [P, KT_H, R], BF16)
    txtp_d = nc.dram_tensor("txtp_spill", [P, KT_H, R], BF16)

    outT = nc.dram_tensor("outT", [H, R], F32, kind="ExternalOutput")

    with tile.TileContext(nc) as tc:
        def open_pool(**kw):
            cm = tc.tile_pool(**kw)
            return cm, cm.__enter__()

        def scope(name):
            import contextlib

            @contextlib.contextmanager
            def _s():
                sid, _ = nc.enter_named_scope(name, False)
                yield
                nc.leave_named_scope(name, sid, False)
            return _s()

        # -------- constants (whole kernel) --------
        const_cm, const = open_pool(name="const", bufs=1)
        ones128 = const.tile([P, P], BF16)
        nc.vector.memset(ones128[:], 1.0)
        eps_col = const.tile([P, 1], F32)
        nc.vector.memset(eps_col[:], EPS)
        zero_col = const.tile([P, 1], F32)
        nc.vector.memset(zero_col[:], 0.0)
        bias_all = const.tile([P, NB], F32)
        bias_sb = {n: bias_all[:, i * KT_H:(i + 1) * KT_H]
                   for i, n in enumerate(bias_names)}
        fb1_sb = bias_all[:, len(bias_names) * KT_H:]
        hmask_sb = const.tile([P, 2], BF16)
        hmaskT_sb = const.tile([2, P], BF16)

        def load_consts():
            nc.sync.dma_start(bias_all[:], bias_all_d[:, :])
            nc.sync.dma_start(hmask_sb[:], hmask_d[:, :])
            nc.sync.dma_start(hmaskT_sb[:], hmaskT_d[:, :])

        # -------- shared SBUF pools (whole kernel) --------
        wpool_cm, wpool = open_pool(name="wpool", bufs=2)   # "w" 4KB slots x2
        tpool_cm, tpool = open_pool(name="tpool", bufs=6)   # "tmp" 2KB x6
        spool_cm, spool = open_pool(name="spool", bufs=4)   # "small" 2KB x4
        acts_cm, acts = open_pool(name="acts", bufs=1)

        def act_tile(tag, name, dt=BF16, pad16=True):
            shape = [P, KT_H, R]
            pad = None
            if dt == FP8 and pad16:
                pad = [P, KT_H, 2 * R]  # keep the recycled slot at 16KB
            return acts.tile(shape, dt, tag=tag, name=name, padded_shape=pad)

        def sp_tile(name, shape=None, dt=BF16):
            """SP slot is 16KB/partition (bf16 [P, KT_H, R])."""
            shape = shape or [P, KT_H, R]
            pad = None
            if mybir.dt.size(dt) == 1:
                pad = [shape[0], shape[1], shape[2] * 2]
            return acts.tile(shape, dt, tag="SP", name=name, padded_shape=pad)

        pmain = None
        paux = None

        def load_w(wT_d, kt, mt, name):
            """bf16 weight slice wT[:, mt*128:(mt+1)*128] as [128, kt, 128]."""
            if kt > KT_H:
                t = wpool.tile([P, KT_F, P], BF16, tag="w", name=name)
            else:
                t = wpool.tile([P, KT_H, P], BF16, tag="w_h", name=name, bufs=6)
            nc.sync.dma_start(
                t[:, :kt, :],
                wT_d[:, mt * P:(mt + 1) * P].rearrange("(k p) m -> p k m", p=P),
            )
            return t

        def load_w8(wT_d, kt, mt, name):
            """fp8 weight slice as [128, kt, 128] (big 'w' slot is 4KB fp8)."""
            if kt > KT_H:
                t = wpool.tile([P, KT_F, P], FP8, tag="w", name=name)
            else:
                t = wpool.tile([P, KT_H, P], FP8, tag="w_h", name=name, bufs=6,
                               padded_shape=[P, KT_H, 2 * P])
            nc.sync.dma_start(
                t[:, :kt, :],
                wT_d[:, mt * P:(mt + 1) * P].rearrange("(k p) m -> p k m", p=P),
            )
            return t

        def mm_layer(srcs, mt, evict, wname="w", chs=range(CH)):
            wts = [load_w(w_d, kt, mt, f"{wname}{i}") for i, (w_d, _, kt) in enumerate(srcs)]
            nk_tot = sum(kt for (_, _, kt) in srcs)
            for ch in chs:
                ps = pmain.tile([P, CHS], F32, tag="mm", name="ps_mm")
                i = 0
                for (w_d, act, kt), wt in zip(srcs, wts):
                    for k in range(kt):
                        nc.tensor.matmul(
                            ps[:], lhsT=wt[:, k, :], rhs=act[:, k, _chsl(ch)],
                            start=(i == 0), stop=(i == nk_tot - 1),
                        )
                        i += 1
                evict(mt, ch, ps)

        def mm_dr(ps, wt, act8, kt, ch, start=True, stop=True):
            """fp8 DoubleRow accumulation chain: kt k-tiles as kt//2 pairs."""
            np_ = kt // 2
            for k in range(np_):
                nc.tensor.matmul(
                    ps[:], lhsT=wt[:, 2 * k:2 * k + 2, :],
                    rhs=act8[:, 2 * k:2 * k + 2, _chsl(ch)],
                    start=(start and k == 0), stop=(stop and k == np_ - 1),
                    perf_mode=DR,
                )

        def evict_bias(dst, bname):
            b = bias_sb[bname]

            def _e(mt, ch, ps):
                nc.scalar.activation(
                    dst[:, mt, _chsl(ch)], ps[:], AF.Identity,
                    bias=b[:, mt:mt + 1], scale=1.0,
                )
            return _e

        def evict_bias_res(dst, bname, res):
            b = bias_sb[bname]

            def _e(mt, ch, ps):
                nc.vector.scalar_tensor_tensor(
                    dst[:, mt, _chsl(ch)], ps[:], b[:, mt:mt + 1],
                    res[:, mt, _chsl(ch)], op0=ALU.add, op1=ALU.add,
                )
            return _e

        lnp_cm, lnp = open_pool(name="lnp", bufs=4)  # LN stats (mf bf16, ivf f32)

        def ln_stats(x_bf, ch):
            """Row mean + rsqrt(var) via ones-matmuls. Emitted for ALL chunks
            before any normalize pass so the x^2 ACT feeds are not queued
            behind the normalize evict ACTs (which stalled the Qb matmuls)."""
            sb_ = paux.tile([P, CHS], F32, tag="Sb", name="ln_Sb")
            for k in range(KT_H):
                nc.tensor.matmul(sb_[:], lhsT=ones128[:],
                                 rhs=x_bf[:, k, _chsl(ch)],
                                 start=(k == 0), stop=(k == KT_H - 1))
            qb_ = paux.tile([P, CHS], F32, tag="Qb", name="ln_Qb")
            for k in range(KT_H):
                x2 = tpool.tile([P, CHS], BF16, tag="tmp", name="ln_x2")
                nc.vector.tensor_mul(out=x2[:], in0=x_bf[:, k, _chsl(ch)],
                                     in1=x_bf[:, k, _chsl(ch)])
                nc.tensor.matmul(qb_[:], lhsT=ones128[:], rhs=x2[:],
                                 start=(k == 0), stop=(k == KT_H - 1))
            mf = lnp.tile([P, CHS], BF16, tag="lnm", name="ln_mf")
            nc.vector.tensor_scalar_mul(mf[:], sb_[:], 1.0 / H)
            msq = tpool.tile([P, CHS], F32, tag="tmp", name="ln_msq")
            nc.vector.tensor_mul(out=msq[:], in0=mf[:], in1=mf[:])
            vf = tpool.tile([P, CHS], F32, tag="tmp", name="ln_vf")
            nc.vector.scalar_tensor_tensor(vf[:], qb_[:], 1.0 / H, msq[:],
                                           op0=ALU.mult, op1=ALU.subtract)
            sd = tpool.tile([P, CHS], F32, tag="tmp", name="ln_sd")
            nc.scalar.activation(sd[:], vf[:], AF.Sqrt, bias=eps_col[:], scale=1.0)
            # ~5x faster than nc.vector.reciprocal (which stalled PE 3.4us)
            ivf_f = tpool.tile([P, CHS], F32, tag="tmp", name="ln_ivf_f")
            nc.vector.reciprocal_approx_fast(out=ivf_f[:], in_=sd[:])
            ivf = lnp.tile([P, CHS], BF16, tag="lni", name="ln_ivf")
            nc.vector.tensor_scalar_mul(ivf[:], ivf_f[:], 1.0)
            return mf, ivf

        def ln_norm(x_bf, stats, ch, g_name, b_name, out_bf,
                    out_f8=None, g8_name=None, b8_name=None, f8_eng=None):
            g = bias_sb[g_name]
            bb = bias_sb[b_name]
            mf, ivf = stats
            for k in range(KT_H):
                t1 = tpool.tile([P, CHS], BF16, tag="tmp", name="ln_t1")
                nc.vector.tensor_sub(out=t1[:], in0=x_bf[:, k, _chsl(ch)], in1=mf[:])
                t2 = tpool.tile([P, CHS], BF16, tag="tmp", name="ln_t2")
                nc.vector.tensor_mul(out=t2[:], in0=t1[:], in1=ivf[:])
                nc.vector.tensor_scalar(out_bf[:, k, _chsl(ch)], t2[:],
                                        g[:, k:k + 1], bb[:, k:k + 1],
                                        op0=ALU.mult, op1=ALU.add)
                if out_f8 is not None:
                    g8 = bias_sb[g8_name]
                    b8 = bias_sb[b8_name]
                    (f8_eng or nc.vector).tensor_scalar(
                        out_f8[:, k, _chsl(ch)], t2[:],
                        g8[:, k:k + 1], b8[:, k:k + 1],
                        op0=ALU.mult, op1=ALU.add)

        def layernorm(x_bf, g_name, b_name, out_bf,
                      out_f8=None, g8_name=None, b8_name=None):
            st = [ln_stats(x_bf, ch) for ch in range(CH)]
            for ch in range(CH):
                ln_norm(x_bf, st[ch], ch, g_name, b_name, out_bf,
                        out_f8, g8_name, b8_name)

        # ================= P0/P1: input projections (streamed) =============
        imgp = act_tile("S1", "imgp")
        txtp = act_tile("S2", "txtp")

        def input_proj(xT_d, w_d, kt_in, bname, dst, post_dma=None):
            for ch in range(CH):
                pss = [pmain.tile([P, CHS], F32, tag=f"mm{mt}", name=f"ps{mt}")
                       for mt in range(KT_H)]
                for k in range(kt_in):
                    wt = wpool.tile([P, H], BF16, tag="w_h", name="wrow", bufs=6)
                    nc.sync.dma_start(wt[:], w_d[k * P:(k + 1) * P, :])
                    xs = tpool.tile([P, CHS], BF16, tag="tmp", name="xslice")
                    nc.sync.dma_start(xs[:], xT_d[k * P:(k + 1) * P, _chsl(ch)])
                    for mt in range(KT_H):
                        nc.tensor.matmul(pss[mt][:], lhsT=wt[:, mt * P:(mt + 1) * P],
                                         rhs=xs[:], start=(k == 0), stop=(k == kt_in - 1))
                if post_dma is not None:
                    post_dma()
                    post_dma = None
                for mt in range(KT_H):
                    nc.scalar.activation(dst[:, mt, _chsl(ch)], pss[mt][:], AF.Identity,
                                         bias=bias_sb[bname][:, mt:mt + 1], scale=1.0)

        with scope("P01"), tc.tile_pool(name="pmm01", bufs=1, space="PSUM") as pmain:
            input_proj(xiT, wiT, KT_I, "bi", imgp, post_dma=load_consts)
            input_proj(xtT, wtT, KT_T, "bt", txtp)
            nc.sync.dma_start(imgp_d[:, :, :], imgp[:])
            nc.sync.dma_start(txtp_d[:, :, :], txtp[:])
            # prefetch P23's first two weight tiles into the big-weight slots
            # (sized to match the 4KB fp8 FFN slices that share the tag)
            wvo_pre = []
            for mt in range(2):
                t = wpool.tile([P, KT_H, P], BF16, tag="w", name=f"wvo_pre{mt}",
                               padded_shape=[P, 2 * KT_H, P])
                nc.sync.dma_start(
                    t[:, :, :],
                    wvoT[:, mt * P:(mt + 1) * P].rearrange("(k p) m -> p k m", p=P))
                wvo_pre.append(t)

        # ============ P2/P3: merged cross-attention + LN ============
        c0 = act_tile("S3", "c0")
        c1 = act_tile("S4", "c1")
        c0_f8 = act_tile("C8a", "c0_f8", FP8, pad16=False)  # c0/8 for DR rhs
        c1_f8 = act_tile("C8b", "c1_f8", FP8, pad16=False)

        with (
            scope("P23"),
            tc.tile_pool(name="pmm23", bufs=4, space="PSUM") as pmain,
            tc.tile_pool(name="paux23", bufs=2, space="PSUM") as paux,
            tc.tile_pool(name="pca", bufs=1) as pca,
        ):
            # GEMMs first (x0 then x1, weights loaded once per mt); both LN
            # chains afterwards so their ACT/DVE tails hide under the gate
            # GEMMs of P3g (PE never waits on LN).
            x0 = sp_tile("x0")
            ev_x0 = evict_bias_res(x0, "bvo", imgp)
            for mt in range(KT_H):
                if mt < 2:
                    for ch in range(CH):
                        ps = pmain.tile([P, CHS], F32, tag="mm", name="ps_mm")
                        for k in range(KT_H):
                            nc.tensor.matmul(ps[:], lhsT=wvo_pre[mt][:, k, :],
                                             rhs=txtp[:, k, _chsl(ch)],
                                             start=(k == 0), stop=(k == KT_H - 1))
                        ev_x0(mt, ch, ps)
                else:
                    mm_layer([(wvoT, txtp, KT_H)], mt, ev_x0, wname="wvo")
            x1 = pca.tile([P, KT_H, R], BF16, tag="x1", name="x1")
            for mt in range(KT_H):
                mm_layer([(wvoT, imgp, KT_H)], mt, evict_bias_res(x1, "bvo", txtp),
                         wname="wvo")
            st0 = [ln_stats(x0, ch) for ch in range(CH)]
            st1 = [ln_stats(x1, ch) for ch in range(CH)]
            for ch in range(CH):
                ln_norm(x0, st0[ch], ch, "n1g", "n1b", c0,
                        c0_f8, "n1g8", "n1b8")
            cd_f8 = sp_tile("cd", dt=FP8)  # (c0-c1)/8, built per tile
            for ch in range(CH):
                ln_norm(x1, st1[ch], ch, "n2g", "n2b", c1,
                        c1_f8, "n2g8", "n2b8")
                for k in range(KT_H):
                    nc.vector.tensor_sub(out=cd_f8[:, k, _chsl(ch)],
                                         in0=c0_f8[:, k, _chsl(ch)],
                                         in1=c1_f8[:, k, _chsl(ch)])

        # ================ P3g: gate logits (independent filler) ==========
        g_sb = act_tile("SG", "g_sb")
        with (
            scope("P3g"),
            tc.tile_pool(name="pmm3g", bufs=8, space="PSUM") as pmain,
        ):
            for mt in range(KT_H):
                wgi = load_w(gwiT, KT_H, mt, "wgi")
                wgt = load_w(gwtT, KT_H, mt, "wgt")
                for ch in range(CH):
                    ps = pmain.tile([P, CHS], F32, tag="mm", name="ps_g")
                    for k in range(KT_H):
                        nc.tensor.matmul(ps[:], lhsT=wgi[:, k, :],
                                         rhs=imgp[:, k, _chsl(ch)],
                                         start=(k == 0), stop=False)
                    for k in range(KT_H):
                        nc.tensor.matmul(ps[:], lhsT=wgt[:, k, :],
                                         rhs=txtp[:, k, _chsl(ch)],
                                         start=False, stop=(k == KT_H - 1))
                    if (mt + ch) % 2 == 0:
                        nc.scalar.activation(g_sb[:, mt, _chsl(ch)], ps[:], AF.Identity,
                                             bias=bias_sb["gb"][:, mt:mt + 1], scale=1.0)
                    else:
                        nc.vector.tensor_scalar(g_sb[:, mt, _chsl(ch)], ps[:],
                                                bias_sb["gb"][:, mt:mt + 1], None,
                                                op0=ALU.add)

        # ============ P4: self-attention qkv + scores (fp8 DR) ============
        v0 = act_tile("S5", "v0")   # stored as v/8 (bf16)
        v1 = act_tile("S6", "v1")
        o0 = act_tile("S1", "o0", FP8)  # o/8 in fp8, after imgp's last read
        o1 = act_tile("S2", "o1", FP8)

        with (
            scope("P4"),
            tc.tile_pool(name="pmm4", bufs=3, space="PSUM") as pmain,
            tc.tile_pool(name="pd", bufs=2, space="PSUM") as pd,
            tc.tile_pool(name="pab", bufs=1, space="PSUM") as pab,
            tc.tile_pool(name="pqk", bufs=1) as pqk,
        ):
            def qkv8(wt, act8, bname, mt, dst_t, dst_mt=None, scale=1.0):
                for ch in range(CH):
                    ps = pmain.tile([P, CHS], F32, tag="mm", name="ps_qkv")
                    mm_dr(ps, wt, act8, KT_H, ch)
                    bias = bias_sb[bname][:, mt:mt + 1] if bname else zero_col[:, :]
                    if dst_mt is None:
                        nc.scalar.activation(dst_t[:, _chsl(ch)], ps[:], AF.Identity,
                                             bias=bias, scale=scale)
                    else:
                        nc.scalar.activation(dst_t[:, dst_mt, _chsl(ch)], ps[:],
                                             AF.Identity, bias=bias, scale=scale)

            hm2 = hmask_sb[:, :]    # [128, 2] local-head one-hot
            hmT2 = hmaskT_sb[:, :]  # [2, 128]
            AB = float(SA / SB)
            for mt in range(KT_H):
                wq_t = load_w8(wqT, KT_H, mt, "wq")
                wv_t = load_w8(wvT, KT_H, mt, "wv")
                wk_t = load_w8(wkT, KT_H, mt, "wk")
                wqb_t = load_w8(wqbT, KT_H, mt, "wqb")
                wvb_t = load_w8(wvbT, KT_H, mt, "wvb")
                q0t = pqk.tile([P, R], BF16, tag="q0t")
                q1t = pqk.tile([P, R], BF16, tag="q1t")
                kdt = pqk.tile([P, R], BF16, tag="kdt")
                qkv8(wq_t, c0_f8, "sbq", mt, q0t)
                qkv8(wv_t, c0_f8, "sbv8", mt, v0, dst_mt=mt, scale=1.0 / 8.0)
                qkv8(wqb_t, c1_f8, "sbq", mt, q1t, scale=AB)
                qkv8(wk_t, cd_f8, None, mt, kdt)  # k0-k1; bias cancels
                nc.vector.tensor_mul(out=q0t[:], in0=q0t[:], in1=kdt[:])
                nc.vector.tensor_mul(out=q1t[:], in0=q1t[:], in1=kdt[:])
                m0, m1 = q0t, q1t
                a_ts = {}
                for ch in range(CH):
                    for m_t, nm in ((m0, "A"), (m1, "B")):
                        dmm = pd.tile([2, CHS], F32, tag="dmm", name=f"dmm{nm}")
                        nc.tensor.matmul(dmm[:], lhsT=hm2, rhs=m_t[:, _chsl(ch)],
                                         start=True, stop=True)
                        a_t = spool.tile([2, CHS], BF16, tag="small", name=f"a{nm}")
                        nc.scalar.activation(a_t[:], dmm[:], AF.Sigmoid,
                                             bias=zero_col[0:2, :],
                                             scale=float(1.0 / np.sqrt(HD)))
                        a_ts[(ch, nm)] = a_t
                qkv8(wvb_t, c1_f8, "sbv8", mt, v1, dst_mt=mt, scale=1.0 / SB)
                for ch in range(CH):
                    diff = tpool.tile([P, CHS], BF16, tag="tmp", name="att_diff")
                    nc.vector.tensor_sub(out=diff[:], in0=v0[:, mt, _chsl(ch)],
                                         in1=v1[:, mt, _chsl(ch)])
                    for o_t, nm in ((o0, "A"), (o1, "B")):
                        ab = pab.tile([P, CHS], F32, tag=f"ab{nm}", name=f"ab{nm}")
                        nc.tensor.matmul(ab[:], lhsT=hmT2, rhs=a_ts[(ch, nm)][:],
                                         start=True, stop=True)
                        t_t = tpool.tile([P, CHS], BF16, tag="tmp", name=f"att_t{nm}")
                        nc.vector.tensor_mul(out=t_t[:], in0=diff[:], in1=ab[:])
                        nc.vector.tensor_add(out=o_t[:, mt, _chsl(ch)], in0=t_t[:],
                                             in1=v1[:, mt, _chsl(ch)])

        # prefetch the first two FFN w1 tiles into the free big-weight slots
        # so P67 starts without a DMA wait
        w1_pre = []
        for mt in range(2):
            t = wpool.tile([P, KT_F, P], FP8, tag="w", name=f"w1_pre{mt}")
            nc.sync.dma_start(
                t[:, :KT_H, :],
                w1aT[:, mt * P:(mt + 1) * P].rearrange("(k p) m -> p k m", p=P))
            w1_pre.append(t)

        # ===== P5: SA out-proj (fp8 DR) + residual + LN3 =====
        # LN3's bf16 output is stored as r + ffn_b2 (bias n3bf = n3_b + fb2) so
        # the FFN w2 evict needs no extra bias op; the fp8 copy holds r/8.
        r0 = act_tile("S1", "r0")    # r0 + fb2 (bf16); reuses o0 slot
        r1 = act_tile("S2", "r1")
        r0_f8 = act_tile("C8a", "r0_f8", FP8, pad16=False)  # reuse c_f8 slots
        r1_f8 = act_tile("C8b", "r1_f8", FP8, pad16=False)
        with (
            scope("P5"),
            tc.tile_pool(name="pmm5", bufs=6, space="PSUM") as pmain,
            tc.tile_pool(name="paux5", bufs=1, space="PSUM") as paux,
            tc.tile_pool(name="psa", bufs=1) as psa,
        ):
            y0 = sp_tile("y0")
            y1 = psa.tile([P, KT_H, R], BF16, tag="y1", name="y1")

            # wo loaded once per mt, both positions; LN3 chains emitted after
            # all GEMMs so their ACT/DVE tails hide under LN3-y1 / P67 PE work.
            for mt in range(KT_H):
                wt = load_w8(woT, KT_H, mt, "wo")
                for o_t, res, dst in ((o0, c0, y0), (o1, c1, y1)):
                    for ch in range(CH):
                        ps = pmain.tile([P, CHS], F32, tag="mm", name="ps_wo")
                        mm_dr(ps, wt, o_t, KT_H, ch)
                        nc.vector.scalar_tensor_tensor(
                            dst[:, mt, _chsl(ch)], ps[:],
                            bias_sb["sbo"][:, mt:mt + 1],
                            res[:, mt, _chsl(ch)], op0=ALU.add, op1=ALU.add)
            sty0 = [ln_stats(y0, ch) for ch in range(CH)]
            for ch in range(CH):
                ln_norm(y0, sty0[ch], ch, "n3gf", "n3bf", r0,
                        r0_f8, "n3g8", "n3b8")
            sty1 = [ln_stats(y1, ch) for ch in range(CH)]
            for ch in range(CH):
                ln_norm(y1, sty1[ch], ch, "n3gf", "n3bf", r1,
                        r1_f8, "n3g8", "n3b8")

        # gate sigmoid in place (ACT is idle here; shortens the P8 tail)
        for mt in range(KT_H):
            for ch in range(CH):
                nc.scalar.activation(g_sb[:, mt, _chsl(ch)], g_sb[:, mt, _chsl(ch)],
                                     AF.Sigmoid, bias=zero_col[:], scale=1.0)

        imgp2 = act_tile("S3", "imgp2")
        txtp2 = act_tile("S4", "txtp2")

        # ===== P6/P7: FFN both positions (fp8 DR); pooled accumulation =====
        # mt-outer / ch-inner: each weight tile is loaded once per position
        # (halves FFN weight DMA); hidden tiles hold the full row range.
        pooled = sp_tile("pooled")  # bf16; pos1 fuses the final combine
        with (
            scope("P67"),
            tc.tile_pool(name="pmm67", bufs=6, space="PSUM") as pmain,
        ):
            gate_done = False
            for pos, (r_f8, r_p, w1d, w2d, first) in enumerate([
                    (r0_f8, r0, w1aT, w2aT, True),
                    (r1_f8, r1, w1bT, w2bT, False)]):
                if not first and not gate_done:
                    # g_sb <- gate*(imgp-txtp) + txtp in place; runs on DVE
                    # during pos0's PE work so the pos1 chain is 3 ops + DMA
                    gate_done = True
                    for gmt in range(KT_H):
                        for gch in range(CH):
                            gsl = _chsl(gch)
                            gd = tpool.tile([P, CHS], BF16, tag="tmp", name="gd")
                            nc.vector.tensor_sub(out=gd[:],
                                                 in0=imgp2[:, gmt, gsl],
                                                 in1=txtp2[:, gmt, gsl])
                            gt = tpool.tile([P, CHS], BF16, tag="tmp", name="gt")
                            nc.vector.tensor_mul(out=gt[:],
                                                 in0=g_sb[:, gmt, gsl], in1=gd[:])
                            nc.vector.tensor_add(out=g_sb[:, gmt, gsl],
                                                 in0=gt[:], in1=txtp2[:, gmt, gsl])
                gelu_scale = 1.0 if first else float(SA / SB)
                ev_scale = (1.0 / SA) if first else (1.0 / SB)
                # hidden [128, 16, 1024] fp8 x2 in the freed v0/v1 slots
                h_a = acts.tile([P, KT_F // 2, R], FP8, tag="S5",
                                name=f"h_a{pos}")
                h_b = acts.tile([P, KT_F // 2, R], FP8, tag="S6",
                                name=f"h_b{pos}")
                for mt in range(KT_F):
                    if first and mt < 2:
                        wt = w1_pre[mt]
                    else:
                        wt = load_w8(w1d, KT_H, mt, "w1")
                    hdst = h_a if mt < KT_F // 2 else h_b
                    for ch in range(CH):
                        ps = pmain.tile([P, CHS], F32, tag="mm", name="ps_f1")
                        mm_dr(ps, wt, r_f8, KT_H, ch)
                        nc.scalar.activation(
                            hdst[:, mt % (KT_F // 2), _chsl(ch)], ps[:],
                            AF.Gelu, bias=fb1_sb[:, mt:mt + 1], scale=gelu_scale)
                if first:
                    # reload the P8 operands here: the 4MB DMA would starve
                    # the w1 weight stream at the P67 head if issued earlier
                    nc.sync.dma_start(imgp2[:], imgp_d[:, :, :])
                    nc.sync.dma_start(txtp2[:], txtp_d[:, :, :])
                for mt in range(KT_H):
                    wt = load_w8(w2d, KT_F, mt, "w2")
                    for ch in range(CH):
                        ps = pmain.tile([P, CHS], F32, tag="mm", name="ps_f2")
                        for k in range(KT_F // 4):
                            nc.tensor.matmul(
                                ps[:], lhsT=wt[:, 2 * k:2 * k + 2, :],
                                rhs=h_a[:, 2 * k:2 * k + 2, _chsl(ch)],
                                start=(k == 0), stop=False, perf_mode=DR)
                        for k in range(KT_F // 4):
                            nc.tensor.matmul(
                                ps[:],
                                lhsT=wt[:, KT_F // 2 + 2 * k:KT_F // 2 + 2 * k + 2, :],
                                rhs=h_b[:, 2 * k:2 * k + 2, _chsl(ch)],
                                start=False, stop=(k == KT_F // 4 - 1),
                                perf_mode=DR)
                        if first:
                            nc.vector.scalar_tensor_tensor(
                                pooled[:, mt, _chsl(ch)], ps[:], ev_scale,
                                r_p[:, mt, _chsl(ch)], op0=ALU.mult, op1=ALU.add)
                        else:
                            # fused final combine: out = 0.5*(pooled0 + p1)
                            #   + g_sb (pre-combined gate term), streamed out
                            tmp = tpool.tile([P, CHS], F32, tag="tmp", name="ffn_tmp")
                            nc.vector.scalar_tensor_tensor(
                                tmp[:], ps[:], ev_scale,
                                r_p[:, mt, _chsl(ch)], op0=ALU.mult, op1=ALU.add)
                            nc.vector.tensor_add(out=tmp[:], in0=tmp[:],
                                                 in1=pooled[:, mt, _chsl(ch)])
                            fin = tpool.tile([P, CHS], F32, tag="tmp", name="gfin")
                            nc.vector.scalar_tensor_tensor(
                                fin[:], tmp[:], 0.5, g_sb[:, mt, _chsl(ch)],
                                op0=ALU.mult, op1=ALU.add)
                            nc.sync.dma_start(outT[mt * P:(mt + 1) * P, _chsl(ch)],
                                              fin[:])

        lnp_cm.__exit__(None, None, None)
        acts_cm.__exit__(None, None, None)
        spool_cm.__exit__(None, None, None)
        tpool_cm.__exit__(None, None, None)
        wpool_cm.__exit__(None, None, None)
        const_cm.__exit__(None, None, None)

    nc.compile()
    return nc


def host_prep(inputs):
    """Host-side preprocessing: merge CA weights, transpose, cast, shard."""
    f = {k: np.asarray(v, dtype=np.float32) for k, v in inputs.items()}

    def bf(x):
        return np.ascontiguousarray(x).astype(np_bf16)

    def q8(x, s):
        return np.ascontiguousarray(np.asarray(x, np.float32) * s).astype(np_fp8)

    def bias128(x, kt):
        return np.ascontiguousarray(np.asarray(x, np.float32).reshape(kt, P).T)

    ca_wv = np.split(f["ca_wqkv"], 3, axis=0)[2]
    ca_bv = f["ca_bqkv"][2 * H:]
    w_vo = f["ca_wo"] @ ca_wv
    b_vo = f["ca_wo"] @ ca_bv + f["ca_bo"]

    sa_wq, sa_wk, sa_wv = np.split(f["sa_wqkv"], 3, axis=0)
    sa_bq, sa_bk, sa_bv = np.split(f["sa_bqkv"], 3)

    gwi = f["gate_w"][:, :H]
    gwt = f["gate_w"][:, H:]

    lh = np.arange(P) // HD  # local head index within a 128-feature tile
    hmask = np.ascontiguousarray((lh[:, None] == np.arange(2)[None, :]).astype(np_bf16))
    hmaskT = np.ascontiguousarray(hmask.T)

    shared = {
        "wiT": bf(f["Wi"].T), "wtT": bf(f["Wt"].T),
        "wvoT": bf(w_vo.T),
        "wqT": q8(sa_wq.T, SA), "wqbT": q8(sa_wq.T, SB),
        "wkT": q8(sa_wk.T, SA),
        "wvT": q8(sa_wv.T, SA), "wvbT": q8(sa_wv.T, SB),
        "woT": q8(f["sa_wo"].T, SA),
        "w1aT": q8(f["ffn_w1"].T, SA), "w1bT": q8(f["ffn_w1"].T, SB),
        "w2aT": q8(f["ffn_w2"].T, SA), "w2bT": q8(f["ffn_w2"].T, SB),
        "gwiT": bf(gwi.T), "gwtT": bf(gwt.T),
        "bias_all": np.concatenate([
            bias128(f["bi"], KT_H), bias128(f["bt"], KT_H), bias128(b_vo, KT_H),
            bias128(sa_bq, KT_H), bias128(sa_bv / 8.0, KT_H),
            bias128(f["sa_bo"], KT_H), bias128(f["ffn_b2"], KT_H),
            bias128(f["gate_b"], KT_H),
            bias128(f["n1_g"], KT_H), bias128(f["n1_b"], KT_H),
            bias128(f["n2_g"], KT_H), bias128(f["n2_b"], KT_H),
            bias128(f["n3_g"], KT_H), bias128(f["n3_b"] + f["ffn_b2"], KT_H),
            bias128(f["n1_g"] / 8.0, KT_H), bias128(f["n1_b"] / 8.0, KT_H),
            bias128(f["n2_g"] / 8.0, KT_H), bias128(f["n2_b"] / 8.0, KT_H),
            bias128(f["n3_g"] / 8.0, KT_H), bias128(f["n3_b"] / 8.0, KT_H),
            bias128(f["ffn_b1"], KT_F),
        ], axis=1),
        "hmask": np.ascontiguousarray(hmask), "hmaskT": hmaskT,
    }

    xiT = f["image_features"].T.astype(np_bf16)  # [IMG_D, B]
    xtT = f["text_features"].T.astype(np_bf16)
    in_maps = []
    for c in range(N_CORES):
        m = dict(shared)
        m["xiT"] = np.ascontiguousarray(xiT[:, c * R:(c + 1) * R])
        m["xtT"] = np.ascontiguousarray(xtT[:, c * R:(c + 1) * R])
        in_maps.append(m)
    return in_maps


_NC_CACHE = None


def kernel(**inputs) -> np.ndarray:
    global _NC_CACHE
    if _NC_CACHE is None:
        _NC_CACHE = build()
    nc = _NC_CACHE
    in_maps = host_prep(inputs)
    res = run_bass_kernel_spmd(nc, in_maps, core_ids=list(range(N_CORES)))
    out = np.empty((B, H), np.float32)
    for c in range(N_CORES):
        out[c * R:(c + 1) * R, :] = res.results[c]["outT"].T
    return out


if __name__ == "__main__":
    nc = build()
    print("built OK")
